# revision 23
# baseline (speedup 1.0000x reference)
"""Multi-head attention (B=4, S=1024, D=1024, H=16) on 8 TRN2 NeuronCores.

Sharding: batch (4) x head-half (2) -> 8 cores, zero cross-core traffic.
Core c handles batch b = c // 2 and heads [hh*8, hh*8+8) where hh = c % 2.
Each core computes a partial output y_part[s, e] (its 512 channels fed
through its slice of Wo) in bf16; the host sums the two partials per batch
in fp32 and adds the bias terms.

v2 schedule (all bf16 matmuls, fp32 accumulation):
  - One strictly-ordered DMA chain on the sync queue: wq0,wk0, xq chunks,
    xk chunks, (xv,wv) chunks, remaining pair weights, wo.  Fine-grained
    per-chunk semaphores let projection matmuls start as chunks land.
  - Pair-0 Q/K projections are DMA-paced during the input load.
  - The 64 (pair, qn, kt) attention iterations run as a flat software
    pipeline: scores (row-tiled K=64 pair of matmuls) -> exp on ACT ->
    lag-2 AV accumulation, with a filler queue feeding the PE idle slots
    (V projection, next-pair Q/K projections, partial O-projection over
    pairs 0-2).  ACT does exp ONLY (normalize moved off it).
  - Normalize: 1/denom via DVE reciprocal_approx_fast on the psO ones-row,
    partition-broadcast + multiply on the Pool engine.
  - Tail: only the cc=3 O-projection matmuls + fused add with the
    cc0-2 partials, stored as bf16.
"""

import os
import sys

sys.path.insert(0, "/opt/trn_rl_repo")

import numpy as np
import ml_dtypes

BF16 = ml_dtypes.bfloat16

B, S, D = 4, 1024, 1024
HEADS = 16
DK = 64
P = 128
NCORES = 8
DCH = D // P       # 8 contraction chunks
PAIRS = 4          # head-pairs per core (8 heads / 2)
QN = 2             # q 512-chunks
KT = 8             # k tiles of 128
VW = 65            # V channels per head + ones column

_STATE = {}


def _build():
    """Build + compile the per-core Bass program (cached)."""
    if "nc" in _STATE:
        return _STATE["nc"]

    import concourse.bass as bass  # noqa: F401
    import concourse.mybir as mybir
    from concourse import bacc
    from concourse import tile

    f32 = mybir.dt.float32
    bf16 = mybir.dt.bfloat16
    AF = mybir.ActivationFunctionType
    ALU = mybir.AluOpType

    # Pin Exp to the one activation table containing it alongside Ln so the
    # table-load pass never alternates tables (each ACT_TABLE_LOAD ~1.3us).
    _orig_tables = bacc.get_activation_tables

    def _pinned_tables(arch):
        t = dict(_orig_tables(arch))
        target = "natural_log_exp_and_others"
        if target in t:
            for k in t:
                if k != target:
                    t[k] = t[k] - {AF.Exp, AF.Ln}
        return t

    bacc.get_activation_tables = _pinned_tables

    nc = bacc.Bacc("TRN2", target_bir_lowering=False, debug=False)

    xq_d = nc.dram_tensor("xq", [D, S], bf16, kind="ExternalInput")
    xk_d = nc.dram_tensor("xk", [D, S], bf16, kind="ExternalInput")
    xv_d = nc.dram_tensor("xv", [D, S], bf16, kind="ExternalInput")
    wq_d = nc.dram_tensor("wq", [PAIRS, D, P], bf16, kind="ExternalInput")
    wk_d = nc.dram_tensor("wk", [PAIRS, D, P], bf16, kind="ExternalInput")
    wv_d = nc.dram_tensor("wv", [D, 512], bf16, kind="ExternalInput")
    wo_d = nc.dram_tensor("wo", [512, D], bf16, kind="ExternalInput")
    bq_d = nc.dram_tensor("bq", [P, PAIRS], f32, kind="ExternalInput")
    bk_d = nc.dram_tensor("bk", [P, PAIRS], f32, kind="ExternalInput")
    mb_d = nc.dram_tensor("mb", [P, KT], f32, kind="ExternalInput")
    y_d = nc.dram_tensor("y", [S, D], bf16, kind="ExternalOutput")

    from contextlib import ExitStack

    with tile.TileContext(nc) as tc, ExitStack() as ctx:
        const = ctx.enter_context(tc.tile_pool(name="const", bufs=1))
        # Resident tensors
        wv_sb = const.tile([P, DCH, 512], bf16)
        xq_sb = const.tile([P, DCH, S], bf16)
        xk_sb = const.tile([P, DCH, S], bf16)
        xv_sb = const.tile([P, DCH, S], bf16)
        wo_sb = const.tile([P, PAIRS, D], bf16)
        v_sb = const.tile([P, KT, 8 * VW], bf16)
        cat_sb = const.tile([P, PAIRS, S], bf16)
        ysb = const.tile([P, KT, D], f32)      # cc0-2 partial O-projection
        bq_sb = const.tile([P, PAIRS], f32)
        bk_sb = const.tile([P, PAIRS], f32)
        mb_sb = const.tile([P, KT], f32)

        # Pools
        wqp = ctx.enter_context(tc.tile_pool(name="wqp", bufs=3))
        wkp = ctx.enter_context(tc.tile_pool(name="wkp", bufs=3))
        qtp = ctx.enter_context(tc.tile_pool(name="qtp", bufs=2))
        ktp = ctx.enter_context(tc.tile_pool(name="ktp", bufs=2))
        epool = ctx.enter_context(tc.tile_pool(name="epool", bufs=14))
        spool = ctx.enter_context(tc.tile_pool(name="spool", bufs=3))
        rpool = ctx.enter_context(tc.tile_pool(name="rpool", bufs=4))
        r2pool = ctx.enter_context(tc.tile_pool(name="r2pool", bufs=3))
        ypool = ctx.enter_context(tc.tile_pool(name="ypool", bufs=3))
        psacc = ctx.enter_context(tc.tile_pool(name="psacc", bufs=2, space="PSUM"))
        pssp = ctx.enter_context(tc.tile_pool(name="pssp", bufs=2, space="PSUM"))
        psop = ctx.enter_context(tc.tile_pool(name="psop", bufs=2, space="PSUM"))

        # --- tiny loads + ones staging (off the main DMA chain) ---
        nc.scalar.dma_start(bq_sb[:], bq_d.ap())
        nc.scalar.dma_start(bk_sb[:], bk_d.ap())
        nc.scalar.dma_start(mb_sb[:], mb_d.ap())
        ones_f32 = const.tile([P, KT, 8], f32)
        nc.vector.memset(ones_f32[:], 1.0)
        ones_view = v_sb.rearrange("p t (h c) -> p t h c", c=VW)[:, :, :, 64:65]
        nc.vector.tensor_copy(ones_view, ones_f32[:].unsqueeze(3))

        # --- the ordered DMA chain (sync queue = strict transfer order) ---
        xq_r = xq_d.ap().rearrange("(d p) s -> d p s", p=P)
        xk_r = xk_d.ap().rearrange("(d p) s -> d p s", p=P)
        xv_r = xv_d.ap().rearrange("(d p) s -> d p s", p=P)
        wv_r = wv_d.ap().rearrange("(d p) m -> d p m", p=P)
        wq_r = wq_d.ap().rearrange("j (d p) m -> j p d m", p=P)
        wk_r = wk_d.ap().rearrange("j (d p) m -> j p d m", p=P)

        # Big-DMA chain on the gpsimd queue (idle until the first broadcast at
        # ~35us), in strict priority order.  Late pair weights + wo go on the
        # vector queue AFTER the pair-0 evicts so their transfers can't steal
        # HBM bandwidth from the critical xq/xk stream.
        wq_t = [None] * PAIRS
        wk_t = [None] * PAIRS
        wq_t[0] = wqp.tile([P, DCH, P], bf16, tag="wq", name="wq0")
        wk_t[0] = wkp.tile([P, DCH, P], bf16, tag="wk", name="wk0")
        nc.gpsimd.dma_start(wq_t[0][:], wq_r[0])
        nc.gpsimd.dma_start(wk_t[0][:], wk_r[0])
        for d in range(DCH):
            nc.gpsimd.dma_start(xq_sb[:, d], xq_r[d])
        for d in range(DCH):
            nc.gpsimd.dma_start(xk_sb[:, d], xk_r[d])
        for j in range(1, PAIRS):
            wq_t[j] = wqp.tile([P, DCH, P], bf16, tag="wq", name=f"wq{j}")
            wk_t[j] = wkp.tile([P, DCH, P], bf16, tag="wk", name=f"wk{j}")
        # wq1/wk1 right after xk so the pair-1 projection fillers can run
        # during pair 0; wq2/wq3/wo trail the xv stream (needed much later).
        nc.gpsimd.dma_start(wq_t[1][:], wq_r[1])
        nc.gpsimd.dma_start(wk_t[1][:], wk_r[1])
        for d in range(DCH):
            nc.gpsimd.dma_start(wv_sb[:, d], wv_r[d])
            nc.gpsimd.dma_start(xv_sb[:, d], xv_r[d])
        for j in range(2, PAIRS):
            nc.gpsimd.dma_start(wq_t[j][:], wq_r[j])
            nc.gpsimd.dma_start(wk_t[j][:], wk_r[j])
        nc.gpsimd.dma_start(wo_sb[:], wo_d.ap().rearrange("(c p) e -> p c e", p=P))

        qt_t = [None] * PAIRS
        kt_t = [None] * PAIRS

        def qk_group(proj, j, qn):
            """Generator: 8 DMA-paced projection matmuls + bias evict."""
            if proj == "q":
                if qt_t[j] is None:
                    qt_t[j] = qtp.tile([P, S], bf16, tag="qt", name=f"qt{j}")
                w, x, dst, b = wq_t[j], xq_sb, qt_t[j], bq_sb
            else:
                if kt_t[j] is None:
                    kt_t[j] = ktp.tile([P, S], bf16, tag="kt", name=f"kt{j}")
                w, x, dst, b = wk_t[j], xk_sb, kt_t[j], bk_sb
            ps = psacc.tile([P, 512], f32, tag="acc", name=f"ps{proj}{j}_{qn}")
            for d in range(DCH):
                nc.tensor.matmul(
                    ps[:],
                    w[:, d],
                    x[:, d, qn * 512 : (qn + 1) * 512],
                    start=(d == 0),
                    stop=(d == DCH - 1),
                )
                yield
            nc.vector.tensor_scalar_add(
                dst[:, qn * 512 : (qn + 1) * 512], ps[:], b[:, j : j + 1]
            )

        v_ready = [False] * KT

        def v_group(st0, nst):
            """Generator: V' projection for st0..st0+nst-1, d-interleaved."""
            ps = [
                psacc.tile([P, 512], f32, tag="acc", name=f"psv{st0 + i}")
                for i in range(nst)
            ]
            for d in range(DCH):
                for i in range(nst):
                    st = st0 + i
                    nc.tensor.matmul(
                        ps[i][:],
                        xv_sb[:, d, st * P : (st + 1) * P],
                        wv_sb[:, d],
                        start=(d == 0),
                        stop=(d == DCH - 1),
                    )
                    yield
            for i in range(nst):
                st = st0 + i
                vview = v_sb[:, st].rearrange("p (h c) -> p h c", c=VW)
                nc.vector.tensor_copy(
                    vview[:, :, 0:64], ps[i][:].rearrange("p (h c) -> p h c", c=64)
                )
                v_ready[st] = True

        def o_partial(st, en):
            """Generator: partial O-projection over pairs 0-1 -> ysb (f32)."""
            ps = psacc.tile([P, 512], f32, tag="acc", name=f"psy1_{st}_{en}")
            for cc in range(2):
                nc.tensor.matmul(
                    ps[:],
                    cat_sb[:, cc, st * P : (st + 1) * P],
                    wo_sb[:, cc, en * 512 : (en + 1) * 512],
                    start=(cc == 0),
                    stop=(cc == 1),
                )
                yield
            nc.vector.tensor_copy(ysb[:, st, en * 512 : (en + 1) * 512], ps[:])

        # Pair-0 Q/K projections: DMA-paced, before the pipeline.
        for gen in (
            qk_group("q", 0, 0),
            qk_group("q", 0, 1),
            qk_group("k", 0, 0),
            qk_group("k", 0, 1),
        ):
            for _ in gen:
                pass



        # Filler queue for the attention pipeline: (min_slot, tag, generator).
        fillers = []
        fillers.append((2, "qk1", qk_group("q", 1, 0)))
        fillers.append((3, "qk1", qk_group("q", 1, 1)))
        fillers.append((2, "v", v_group(0, 2)))   # DMA-paced by xv arrival
        fillers.append((4, "v", v_group(2, 2)))
        fillers.append((6, "v", v_group(4, 2)))
        fillers.append((8, "v", v_group(6, 2)))
        fillers.append((8, "qk1", qk_group("k", 1, 0)))
        fillers.append((8, "qk1", qk_group("k", 1, 1)))
        for qn in range(QN):
            fillers.append((16 + 4 * qn, "qk2", qk_group("q", 2, qn)))
            fillers.append((20 + 4 * qn, "qk2", qk_group("k", 2, qn)))
        for qn in range(QN):
            fillers.append((32 + 4 * qn, "qk3", qk_group("q", 3, qn)))
            fillers.append((36 + 4 * qn, "qk3", qk_group("k", 3, qn)))
        for st in range(KT):
            for en in range(2):
                fillers.append((33, "op", o_partial(st, en)))

        y_r = y_d.ap().rearrange("(st p) e -> st p e", p=P)
        tail_n = [0]

        def o_final(st, en):
            """cc=2,3 O-matmuls + fused add with the cc0-1 partial + store."""
            i = tail_n[0]
            tail_n[0] += 1
            # During the pipeline (first half) stay off the scores pool —
            # stealing pssp there stalls the final pair's score tiles.
            if i < 8 or i % 4 < 2:
                ps = psacc.tile([P, 512], f32, tag="acc", name=f"psy2_{st}_{en}")[:]
            else:
                if i % 4 == 2:
                    o_final.pt = pssp.tile([P, 2, 512], f32, tag="s", name=f"psy2p_{st}_{en}")
                ps = o_final.pt[:, i % 2]
            for cc in range(2, 4):
                nc.tensor.matmul(
                    ps,
                    cat_sb[:, cc, st * P : (st + 1) * P],
                    wo_sb[:, cc, en * 512 : (en + 1) * 512],
                    start=(cc == 2),
                    stop=(cc == 3),
                )
                yield
            y2 = ypool.tile([P, 512], bf16, tag="y", name=f"y{st}_{en}")
            nc.vector.tensor_tensor(
                y2[:], ps, ysb[:, st, en * 512 : (en + 1) * 512], op=ALU.add
            )
            nc.sync.dma_start(y_r[st][:, en * 512 : (en + 1) * 512], y2[:])

        # First half of the cc2/cc3 tail only needs cat3's qn0 range (s < 512),
        # whose normalize is emitted at slot 57 — run it as fillers.
        for st in range(4):
            for en in range(2):
                fillers.append((57, "t1", o_final(st, en)))
        total_filler_steps = 12 * 8 + 4 * 16 + 16 * 2 + 8 * 4  # MM emissions

        def pop_fillers(slot, budget):
            done = 0
            while done < budget and fillers:
                idx = next(
                    (i for i, (ms, _, _) in enumerate(fillers) if ms <= slot), None
                )
                if idx is None:
                    return done
                try:
                    next(fillers[idx][2])
                    done += 1
                except StopIteration:
                    fillers.pop(idx)
            return done

        def force_drain(tag):
            for entry in [f for f in fillers if f[1] == tag]:
                try:
                    while True:
                        next(entry[2])
                except StopIteration:
                    pass
                fillers.remove(entry)

        # --- the flat attention pipeline: 64 (j, qn, kt) iterations ---
        def emit_scores(j, qn, kt):
            pss = pssp.tile([P, 2, 512], f32, tag="s", name=f"pss{j}_{qn}_{kt}")
            for sub in range(2):
                lo, hi = sub * 64, (sub + 1) * 64
                nc.tensor.matmul(
                    pss[:, sub],
                    kt_t[j][lo:hi, kt * P : (kt + 1) * P],
                    qt_t[j][lo:hi, qn * 512 : (qn + 1) * 512],
                    start=True,
                    stop=True,
                )
            et = epool.tile([P, 2, 512], bf16, tag="e", name=f"e{j}_{qn}_{kt}")
            nc.scalar.activation(
                et[:], pss[:], AF.Exp, bias=mb_sb[:, kt : kt + 1], scale=1.0
            )
            return et

        pso_cur = {}

        def emit_av(j, qn, kt, et):
            for sub in range(2):
                h = j * 2 + sub
                if kt == 0:
                    pso_cur[sub] = psop.tile(
                        [VW, 512], f32, tag="o", name=f"pso{j}_{qn}_{sub}"
                    )
                nc.tensor.matmul(
                    pso_cur[sub][:],
                    v_sb[:, kt, h * VW : (h + 1) * VW],
                    et[:, sub],
                    start=(kt == 0),
                    stop=(kt == KT - 1),
                )

        def emit_normalize(j, qn):
            for sub in range(2):
                lo, hi = sub * 64, (sub + 1) * 64
                pso = pso_cur[sub]
                stg = spool.tile([64, 512], f32, tag="stg", name=f"stg{j}_{qn}_{sub}")
                nc.vector.tensor_copy(stg[:], pso[0:64, :])
                # 1/denom on DVE.  NOTE (HW-verified): reciprocal_approx_fast
                # silently corrupts unless its source sits at partition 0 in
                # SBUF, so the ones-row bounces through a partition-0 tile.
                # gpsimd runs ONLY partition_broadcast so its custom-op library
                # loads once (builtin ops there thrash LOAD_LIB, ~6.5us/swap).
                den = rpool.tile([1, 512], f32, tag="d", name=f"d{j}_{qn}_{sub}")
                nc.vector.tensor_copy(den[:], pso[64:65, :])
                rrow = rpool.tile([1, 512], f32, tag="r", name=f"r{j}_{qn}_{sub}")
                nc.vector.reciprocal_approx_fast(rrow[:], den[:])
                r2 = r2pool.tile([64, 512], f32, tag="r2", name=f"r2{j}_{qn}_{sub}")
                nc.gpsimd.partition_broadcast(r2[:], rrow[:])
                nc.vector.tensor_tensor(
                    cat_sb[lo:hi, j, qn * 512 : (qn + 1) * 512],
                    stg[0:64, :],
                    r2[:],
                    op=ALU.mult,
                )

        iters = [(j, qn, kt) for j in range(PAIRS) for qn in range(QN) for kt in range(KT)]
        pending = []  # (j, qn, kt, et) awaiting AV emission (lag-2)
        slot = 0
        remaining_steps = total_filler_steps

        def av_drain(n):
            # Pop pending AVs (in order) down to n, but never emit a pair-0 AV
            # before its v_sb k-tile write has been emitted (program-order RAW).
            while len(pending) > n:
                jj, qq, kk, ee = pending[0]
                if jj == 0 and not v_ready[kk]:
                    return
                pending.pop(0)
                emit_av(jj, qq, kk, ee)
                if kk == KT - 1:
                    emit_normalize(jj, qq)

        for j, qn, kt in iters:
            if j >= 1 and qn == 0 and kt == 0:
                force_drain(f"qk{j}")  # qt/kt writes must precede the reads
            et = emit_scores(j, qn, kt)
            pending.append((j, qn, kt, et))
            av_drain(2)
            budget = max(2, -(-remaining_steps // max(1, 64 - slot)))
            if len(pending) > 4:  # pair-0 backlog: push V emission along
                budget += len(pending) - 4
            remaining_steps -= pop_fillers(slot, budget)
            slot += 1
        av_drain(0)
        while fillers:
            if pop_fillers(10 ** 9, 1 << 30) == 0:
                break

        # --- tail: cc=3 O-projection for the remaining s-tiles ---
        for st in range(4, KT):
            for en in range(2):
                for _ in o_final(st, en):
                    pass

    nc.compile()
    _STATE["nc"] = nc
    return nc


def _shard(q, k, v, mask, Wq, bq, Wk, bk, Wv, bv, Wo, bo):
    """Build the 8 per-core input maps (host-side layout preparation)."""
    scale = 1.0 / np.sqrt(DK)
    in_maps = []
    for c in range(NCORES):
        b = c // 2
        hh = c % 2
        c0 = hh * 512
        wq_s = (Wq[c0 : c0 + 512, :] * scale).T  # [D, 512]
        wk_s = Wk[c0 : c0 + 512, :].T
        wv_s = Wv[c0 : c0 + 512, :].T
        wo_s = Wo[:, c0 : c0 + 512].T  # [512, D]
        mrow = mask[b, 0, 0, :]
        in_maps.append(
            {
                "xq": np.ascontiguousarray(q[b].T).astype(BF16),
                "xk": np.ascontiguousarray(k[b].T).astype(BF16),
                "xv": np.ascontiguousarray(v[b].T).astype(BF16),
                "wq": np.ascontiguousarray(
                    wq_s.reshape(D, PAIRS, P).transpose(1, 0, 2)
                ).astype(BF16),
                "wk": np.ascontiguousarray(
                    wk_s.reshape(D, PAIRS, P).transpose(1, 0, 2)
                ).astype(BF16),
                "wv": np.ascontiguousarray(wv_s).astype(BF16),
                "wo": np.ascontiguousarray(wo_s).astype(BF16),
                "bq": np.ascontiguousarray(
                    (bq[c0 : c0 + 512] * scale).reshape(PAIRS, P).T, dtype=np.float32
                ),
                "bk": np.ascontiguousarray(
                    bk[c0 : c0 + 512].reshape(PAIRS, P).T, dtype=np.float32
                ),
                "mb": np.ascontiguousarray(
                    np.where(mrow == 0, np.float32(-1e9), np.float32(0.0))
                    .astype(np.float32)
                    .reshape(KT, P)
                    .T
                ),
            }
        )
    return in_maps


def _gather(results, Wv, bv, Wo, bo):
    """Sum per-core partials into the full [B, S, D] output."""
    # Channel-bias correction folded out of the device kernel: the V bias
    # passes through softmax-weighted sums with total weight 1, so its
    # contribution to y is the constant row Wo @ bv.
    corr = (Wo.astype(np.float64) @ bv.astype(np.float64)).astype(np.float32)
    y = np.empty((B, S, D), dtype=np.float32)
    for b in range(B):
        y[b] = (
            results[2 * b]["y"].astype(np.float32)
            + results[2 * b + 1]["y"].astype(np.float32)
            + corr
            + bo
        )
    return y


def _run(trace=False, **inputs):
    import time

    from concourse.bass_utils import run_bass_kernel_spmd

    nc = _build()
    args = {k: np.asarray(v) for k, v in inputs.items()}
    in_maps = _shard(**args)
    last_err = None
    for attempt in range(3):
        try:
            res = run_bass_kernel_spmd(
                nc, in_maps, core_ids=list(range(NCORES)), trace=trace
            )
            break
        except Exception as e:  # device occasionally wedges; retry recovers
            last_err = e
            time.sleep(10 * (attempt + 1))
    else:
        raise last_err
    y = _gather(res.results, args["Wv"], args["bv"], args["Wo"], args["bo"])
    return y, res


def kernel(**inputs):
    y, _ = _run(trace=False, **inputs)
    return y


# revision 24
# speedup vs baseline: 1.0088x; 1.0088x over previous
"""Multi-head attention (B=4, S=1024, D=1024, H=16) on 8 TRN2 NeuronCores.

Sharding: batch (4) x head-half (2) -> 8 cores, zero cross-core traffic.
Core c handles batch b = c // 2 and heads [hh*8, hh*8+8) where hh = c % 2.
Each core computes a partial output y_part[s, e] (its 512 channels fed
through its slice of Wo) in bf16; the host sums the two partials per batch
in fp32 and adds the bias terms.

v2 schedule (all bf16 matmuls, fp32 accumulation):
  - One strictly-ordered DMA chain on the sync queue: wq0,wk0, xq chunks,
    xk chunks, (xv,wv) chunks, remaining pair weights, wo.  Fine-grained
    per-chunk semaphores let projection matmuls start as chunks land.
  - Pair-0 Q/K projections are DMA-paced during the input load.
  - The 64 (pair, qn, kt) attention iterations run as a flat software
    pipeline: scores (row-tiled K=64 pair of matmuls) -> exp on ACT ->
    lag-2 AV accumulation, with a filler queue feeding the PE idle slots
    (V projection, next-pair Q/K projections, partial O-projection over
    pairs 0-2).  ACT does exp ONLY (normalize moved off it).
  - Normalize: 1/denom via DVE reciprocal_approx_fast on the psO ones-row,
    partition-broadcast + multiply on the Pool engine.
  - Tail: only the cc=3 O-projection matmuls + fused add with the
    cc0-2 partials, stored as bf16.
"""

import os
import sys

sys.path.insert(0, "/opt/trn_rl_repo")

import numpy as np
import ml_dtypes

BF16 = ml_dtypes.bfloat16

B, S, D = 4, 1024, 1024
HEADS = 16
DK = 64
P = 128
NCORES = 8
DCH = D // P       # 8 contraction chunks
PAIRS = 4          # head-pairs per core (8 heads / 2)
QN = 2             # q 512-chunks
KT = 8             # k tiles of 128
VW = 65            # V channels per head + ones column

_STATE = {}


def _build():
    """Build + compile the per-core Bass program (cached)."""
    if "nc" in _STATE:
        return _STATE["nc"]

    import concourse.bass as bass  # noqa: F401
    import concourse.mybir as mybir
    from concourse import bacc
    from concourse import tile

    f32 = mybir.dt.float32
    bf16 = mybir.dt.bfloat16
    AF = mybir.ActivationFunctionType
    ALU = mybir.AluOpType

    # Pin Exp to the one activation table containing it alongside Ln so the
    # table-load pass never alternates tables (each ACT_TABLE_LOAD ~1.3us).
    _orig_tables = bacc.get_activation_tables

    def _pinned_tables(arch):
        t = dict(_orig_tables(arch))
        target = "natural_log_exp_and_others"
        if target in t:
            for k in t:
                if k != target:
                    t[k] = t[k] - {AF.Exp, AF.Ln}
        return t

    bacc.get_activation_tables = _pinned_tables

    nc = bacc.Bacc("TRN2", target_bir_lowering=False, debug=False)

    xq_d = nc.dram_tensor("xq", [D, S], bf16, kind="ExternalInput")
    xk_d = nc.dram_tensor("xk", [D, S], bf16, kind="ExternalInput")
    xv_d = nc.dram_tensor("xv", [D, S], bf16, kind="ExternalInput")
    wq_d = nc.dram_tensor("wq", [PAIRS, D, P], bf16, kind="ExternalInput")
    wk_d = nc.dram_tensor("wk", [PAIRS, D, P], bf16, kind="ExternalInput")
    wv_d = nc.dram_tensor("wv", [D, 512], bf16, kind="ExternalInput")
    wo_d = nc.dram_tensor("wo", [512, D], bf16, kind="ExternalInput")
    bq_d = nc.dram_tensor("bq", [P, PAIRS], f32, kind="ExternalInput")
    bk_d = nc.dram_tensor("bk", [P, PAIRS], f32, kind="ExternalInput")
    mb_d = nc.dram_tensor("mb", [P, KT], f32, kind="ExternalInput")
    y_d = nc.dram_tensor("y", [S, D], bf16, kind="ExternalOutput")

    from contextlib import ExitStack

    with tile.TileContext(nc) as tc, ExitStack() as ctx:
        const = ctx.enter_context(tc.tile_pool(name="const", bufs=1))
        # Resident tensors
        wv_sb = const.tile([P, DCH, 512], bf16)
        xq_sb = const.tile([P, DCH, S], bf16)
        xk_sb = const.tile([P, DCH, S], bf16)
        xv_sb = const.tile([P, DCH, S], bf16)
        wo_sb = const.tile([P, PAIRS, D], bf16)
        v_sb = const.tile([P, KT, 8 * VW], bf16)
        cat_sb = const.tile([P, PAIRS, S], bf16)
        bq_sb = const.tile([P, PAIRS], f32)
        bk_sb = const.tile([P, PAIRS], f32)
        mb_sb = const.tile([P, KT], f32)

        # Pools
        wqp = ctx.enter_context(tc.tile_pool(name="wqp", bufs=3))
        wkp = ctx.enter_context(tc.tile_pool(name="wkp", bufs=3))
        qtp = ctx.enter_context(tc.tile_pool(name="qtp", bufs=2))
        ktp = ctx.enter_context(tc.tile_pool(name="ktp", bufs=2))
        epool = ctx.enter_context(tc.tile_pool(name="epool", bufs=14))
        spool = ctx.enter_context(tc.tile_pool(name="spool", bufs=3))
        rpool = ctx.enter_context(tc.tile_pool(name="rpool", bufs=4))
        r2pool = ctx.enter_context(tc.tile_pool(name="r2pool", bufs=3))
        ypool = ctx.enter_context(tc.tile_pool(name="ypool", bufs=3))
        psacc = ctx.enter_context(tc.tile_pool(name="psacc", bufs=2, space="PSUM"))
        pssp = ctx.enter_context(tc.tile_pool(name="pssp", bufs=2, space="PSUM"))
        psop = ctx.enter_context(tc.tile_pool(name="psop", bufs=2, space="PSUM"))

        # --- tiny loads + ones staging (off the main DMA chain) ---
        nc.scalar.dma_start(bq_sb[:], bq_d.ap())
        nc.scalar.dma_start(bk_sb[:], bk_d.ap())
        nc.scalar.dma_start(mb_sb[:], mb_d.ap())
        ones_f32 = const.tile([P, KT, 8], f32)
        nc.vector.memset(ones_f32[:], 1.0)
        ones_view = v_sb.rearrange("p t (h c) -> p t h c", c=VW)[:, :, :, 64:65]
        nc.vector.tensor_copy(ones_view, ones_f32[:].unsqueeze(3))

        # --- the ordered DMA chain (sync queue = strict transfer order) ---
        xq_r = xq_d.ap().rearrange("(d p) s -> d p s", p=P)
        xk_r = xk_d.ap().rearrange("(d p) s -> d p s", p=P)
        xv_r = xv_d.ap().rearrange("(d p) s -> d p s", p=P)
        wv_r = wv_d.ap().rearrange("(d p) m -> d p m", p=P)
        wq_r = wq_d.ap().rearrange("j (d p) m -> j p d m", p=P)
        wk_r = wk_d.ap().rearrange("j (d p) m -> j p d m", p=P)

        # Big-DMA chain on the gpsimd queue (idle until the first broadcast at
        # ~35us), in strict priority order.  Late pair weights + wo go on the
        # vector queue AFTER the pair-0 evicts so their transfers can't steal
        # HBM bandwidth from the critical xq/xk stream.
        wq_t = [None] * PAIRS
        wk_t = [None] * PAIRS
        wq_t[0] = wqp.tile([P, DCH, P], bf16, tag="wq", name="wq0")
        wk_t[0] = wkp.tile([P, DCH, P], bf16, tag="wk", name="wk0")
        nc.gpsimd.dma_start(wq_t[0][:], wq_r[0])
        nc.gpsimd.dma_start(wk_t[0][:], wk_r[0])
        for d in range(DCH):
            nc.gpsimd.dma_start(xq_sb[:, d], xq_r[d])
        for d in range(DCH):
            nc.gpsimd.dma_start(xk_sb[:, d], xk_r[d])
        for j in range(1, PAIRS):
            wq_t[j] = wqp.tile([P, DCH, P], bf16, tag="wq", name=f"wq{j}")
            wk_t[j] = wkp.tile([P, DCH, P], bf16, tag="wk", name=f"wk{j}")
        # wq1/wk1 right after xk so the pair-1 projection fillers can run
        # during pair 0; wq2/wq3/wo trail the xv stream (needed much later).
        nc.gpsimd.dma_start(wq_t[1][:], wq_r[1])
        nc.gpsimd.dma_start(wk_t[1][:], wk_r[1])
        for d in range(DCH):
            nc.gpsimd.dma_start(wv_sb[:, d], wv_r[d])
            nc.gpsimd.dma_start(xv_sb[:, d], xv_r[d])
        for j in range(2, PAIRS):
            nc.gpsimd.dma_start(wq_t[j][:], wq_r[j])
            nc.gpsimd.dma_start(wk_t[j][:], wk_r[j])
        nc.gpsimd.dma_start(wo_sb[:], wo_d.ap().rearrange("(c p) e -> p c e", p=P))

        qt_t = [None] * PAIRS
        kt_t = [None] * PAIRS

        def qk_group(proj, j, qn):
            """Generator: 8 DMA-paced projection matmuls + bias evict."""
            if proj == "q":
                if qt_t[j] is None:
                    qt_t[j] = qtp.tile([P, S], bf16, tag="qt", name=f"qt{j}")
                w, x, dst, b = wq_t[j], xq_sb, qt_t[j], bq_sb
            else:
                if kt_t[j] is None:
                    kt_t[j] = ktp.tile([P, S], bf16, tag="kt", name=f"kt{j}")
                w, x, dst, b = wk_t[j], xk_sb, kt_t[j], bk_sb
            ps = psacc.tile([P, 512], f32, tag="acc", name=f"ps{proj}{j}_{qn}")
            for d in range(DCH):
                nc.tensor.matmul(
                    ps[:],
                    w[:, d],
                    x[:, d, qn * 512 : (qn + 1) * 512],
                    start=(d == 0),
                    stop=(d == DCH - 1),
                )
                yield
            nc.vector.tensor_scalar_add(
                dst[:, qn * 512 : (qn + 1) * 512], ps[:], b[:, j : j + 1]
            )

        v_ready = [False] * KT

        def v_group(st0, nst):
            """Generator: V' projection for st0..st0+nst-1, d-interleaved."""
            ps = [
                psacc.tile([P, 512], f32, tag="acc", name=f"psv{st0 + i}")
                for i in range(nst)
            ]
            for d in range(DCH):
                for i in range(nst):
                    st = st0 + i
                    nc.tensor.matmul(
                        ps[i][:],
                        xv_sb[:, d, st * P : (st + 1) * P],
                        wv_sb[:, d],
                        start=(d == 0),
                        stop=(d == DCH - 1),
                    )
                    yield
            for i in range(nst):
                st = st0 + i
                vview = v_sb[:, st].rearrange("p (h c) -> p h c", c=VW)
                nc.vector.tensor_copy(
                    vview[:, :, 0:64], ps[i][:].rearrange("p (h c) -> p h c", c=64)
                )
                v_ready[st] = True

        # Pair-0 Q/K projections: DMA-paced, before the pipeline.
        for gen in (
            qk_group("q", 0, 0),
            qk_group("q", 0, 1),
            qk_group("k", 0, 0),
            qk_group("k", 0, 1),
        ):
            for _ in gen:
                pass



        # Filler queue for the attention pipeline: (min_slot, tag, generator).
        fillers = []
        fillers.append((2, "qk1", qk_group("q", 1, 0)))
        fillers.append((3, "qk1", qk_group("q", 1, 1)))
        fillers.append((2, "v", v_group(0, 2)))   # DMA-paced by xv arrival
        fillers.append((4, "v", v_group(2, 2)))
        fillers.append((6, "v", v_group(4, 2)))
        fillers.append((8, "v", v_group(6, 2)))
        fillers.append((8, "qk1", qk_group("k", 1, 0)))
        fillers.append((8, "qk1", qk_group("k", 1, 1)))
        for qn in range(QN):
            fillers.append((16 + 4 * qn, "qk2", qk_group("q", 2, qn)))
            fillers.append((20 + 4 * qn, "qk2", qk_group("k", 2, qn)))
        for qn in range(QN):
            fillers.append((32 + 4 * qn, "qk3", qk_group("q", 3, qn)))
            fillers.append((36 + 4 * qn, "qk3", qk_group("k", 3, qn)))
        y_r = y_d.ap().rearrange("(st p) e -> st p e", p=P)
        tail_n = [0]

        def o_final(st, en):
            """Full O-projection group (cc0-3) + copy evict + store."""
            i = tail_n[0]
            tail_n[0] += 1
            # During the pipeline (first half) stay off the scores pool —
            # stealing pssp there stalls the final pair's score tiles.
            if i < 8 or i % 4 < 2:
                ps = psacc.tile([P, 512], f32, tag="acc", name=f"psy2_{st}_{en}")[:]
            else:
                if i % 4 == 2:
                    o_final.pt = pssp.tile([P, 2, 512], f32, tag="s", name=f"psy2p_{st}_{en}")
                ps = o_final.pt[:, i % 2]
            for cc in range(PAIRS):
                nc.tensor.matmul(
                    ps,
                    cat_sb[:, cc, st * P : (st + 1) * P],
                    wo_sb[:, cc, en * 512 : (en + 1) * 512],
                    start=(cc == 0),
                    stop=(cc == PAIRS - 1),
                )
                yield
            y2 = ypool.tile([P, 512], bf16, tag="y", name=f"y{st}_{en}")
            nc.vector.tensor_copy(y2[:], ps)
            nc.sync.dma_start(y_r[st][:, en * 512 : (en + 1) * 512], y2[:])

        # First half of the cc2/cc3 tail only needs cat3's qn0 range (s < 512),
        # whose normalize is emitted at slot 57 — run it as fillers.
        for st in range(4):
            for en in range(2):
                fillers.append((57, "t1", o_final(st, en)))
        total_filler_steps = 12 * 8 + 4 * 16 + 8 * 4  # MM emissions

        def pop_fillers(slot, budget):
            done = 0
            while done < budget and fillers:
                idx = next(
                    (i for i, (ms, _, _) in enumerate(fillers) if ms <= slot), None
                )
                if idx is None:
                    return done
                try:
                    next(fillers[idx][2])
                    done += 1
                except StopIteration:
                    fillers.pop(idx)
            return done

        def force_drain(tag):
            for entry in [f for f in fillers if f[1] == tag]:
                try:
                    while True:
                        next(entry[2])
                except StopIteration:
                    pass
                fillers.remove(entry)

        # --- the flat attention pipeline: 64 (j, qn, kt) iterations ---
        def emit_scores(j, qn, kt):
            pss = pssp.tile([P, 2, 512], f32, tag="s", name=f"pss{j}_{qn}_{kt}")
            for sub in range(2):
                lo, hi = sub * 64, (sub + 1) * 64
                nc.tensor.matmul(
                    pss[:, sub],
                    kt_t[j][lo:hi, kt * P : (kt + 1) * P],
                    qt_t[j][lo:hi, qn * 512 : (qn + 1) * 512],
                    start=True,
                    stop=True,
                )
            et = epool.tile([P, 2, 512], bf16, tag="e", name=f"e{j}_{qn}_{kt}")
            nc.scalar.activation(
                et[:], pss[:], AF.Exp, bias=mb_sb[:, kt : kt + 1], scale=1.0
            )
            return et

        pso_cur = {}

        def emit_av(j, qn, kt, et):
            for sub in range(2):
                h = j * 2 + sub
                if kt == 0:
                    pso_cur[sub] = psop.tile(
                        [VW, 512], f32, tag="o", name=f"pso{j}_{qn}_{sub}"
                    )
                nc.tensor.matmul(
                    pso_cur[sub][:],
                    v_sb[:, kt, h * VW : (h + 1) * VW],
                    et[:, sub],
                    start=(kt == 0),
                    stop=(kt == KT - 1),
                )

        def emit_normalize(j, qn):
            for sub in range(2):
                lo, hi = sub * 64, (sub + 1) * 64
                pso = pso_cur[sub]
                stg = spool.tile([64, 512], f32, tag="stg", name=f"stg{j}_{qn}_{sub}")
                nc.vector.tensor_copy(stg[:], pso[0:64, :])
                # 1/denom on DVE.  NOTE (HW-verified): reciprocal_approx_fast
                # silently corrupts unless its source sits at partition 0 in
                # SBUF, so the ones-row bounces through a partition-0 tile.
                # gpsimd runs ONLY partition_broadcast so its custom-op library
                # loads once (builtin ops there thrash LOAD_LIB, ~6.5us/swap).
                den = rpool.tile([1, 512], f32, tag="d", name=f"d{j}_{qn}_{sub}")
                nc.vector.tensor_copy(den[:], pso[64:65, :])
                rrow = rpool.tile([1, 512], f32, tag="r", name=f"r{j}_{qn}_{sub}")
                nc.vector.reciprocal_approx_fast(rrow[:], den[:])
                r2 = r2pool.tile([64, 512], f32, tag="r2", name=f"r2{j}_{qn}_{sub}")
                nc.gpsimd.partition_broadcast(r2[:], rrow[:])
                nc.vector.tensor_tensor(
                    cat_sb[lo:hi, j, qn * 512 : (qn + 1) * 512],
                    stg[0:64, :],
                    r2[:],
                    op=ALU.mult,
                )

        iters = [(j, qn, kt) for j in range(PAIRS) for qn in range(QN) for kt in range(KT)]
        pending = []  # (j, qn, kt, et) awaiting AV emission (lag-2)
        slot = 0
        remaining_steps = total_filler_steps

        def av_drain(n):
            # Pop pending AVs (in order) down to n, but never emit a pair-0 AV
            # before its v_sb k-tile write has been emitted (program-order RAW).
            while len(pending) > n:
                jj, qq, kk, ee = pending[0]
                if jj == 0 and not v_ready[kk]:
                    return
                pending.pop(0)
                emit_av(jj, qq, kk, ee)
                if kk == KT - 1:
                    emit_normalize(jj, qq)

        for j, qn, kt in iters:
            if j >= 1 and qn == 0 and kt == 0:
                force_drain(f"qk{j}")  # qt/kt writes must precede the reads
            et = emit_scores(j, qn, kt)
            pending.append((j, qn, kt, et))
            av_drain(2)
            budget = max(2, -(-remaining_steps // max(1, 64 - slot)))
            if len(pending) > 4:  # pair-0 backlog: push V emission along
                budget += len(pending) - 4
            remaining_steps -= pop_fillers(slot, budget)
            slot += 1
        av_drain(0)
        while fillers:
            if pop_fillers(10 ** 9, 1 << 30) == 0:
                break

        # --- tail: cc=3 O-projection for the remaining s-tiles ---
        for st in range(4, KT):
            for en in range(2):
                for _ in o_final(st, en):
                    pass

    nc.compile()
    _STATE["nc"] = nc
    return nc


def _shard(q, k, v, mask, Wq, bq, Wk, bk, Wv, bv, Wo, bo):
    """Build the 8 per-core input maps (host-side layout preparation)."""
    scale = 1.0 / np.sqrt(DK)
    in_maps = []
    for c in range(NCORES):
        b = c // 2
        hh = c % 2
        c0 = hh * 512
        wq_s = (Wq[c0 : c0 + 512, :] * scale).T  # [D, 512]
        wk_s = Wk[c0 : c0 + 512, :].T
        wv_s = Wv[c0 : c0 + 512, :].T
        wo_s = Wo[:, c0 : c0 + 512].T  # [512, D]
        mrow = mask[b, 0, 0, :]
        in_maps.append(
            {
                "xq": np.ascontiguousarray(q[b].T).astype(BF16),
                "xk": np.ascontiguousarray(k[b].T).astype(BF16),
                "xv": np.ascontiguousarray(v[b].T).astype(BF16),
                "wq": np.ascontiguousarray(
                    wq_s.reshape(D, PAIRS, P).transpose(1, 0, 2)
                ).astype(BF16),
                "wk": np.ascontiguousarray(
                    wk_s.reshape(D, PAIRS, P).transpose(1, 0, 2)
                ).astype(BF16),
                "wv": np.ascontiguousarray(wv_s).astype(BF16),
                "wo": np.ascontiguousarray(wo_s).astype(BF16),
                "bq": np.ascontiguousarray(
                    (bq[c0 : c0 + 512] * scale).reshape(PAIRS, P).T, dtype=np.float32
                ),
                "bk": np.ascontiguousarray(
                    bk[c0 : c0 + 512].reshape(PAIRS, P).T, dtype=np.float32
                ),
                "mb": np.ascontiguousarray(
                    np.where(mrow == 0, np.float32(-1e9), np.float32(0.0))
                    .astype(np.float32)
                    .reshape(KT, P)
                    .T
                ),
            }
        )
    return in_maps


def _gather(results, Wv, bv, Wo, bo):
    """Sum per-core partials into the full [B, S, D] output."""
    # Channel-bias correction folded out of the device kernel: the V bias
    # passes through softmax-weighted sums with total weight 1, so its
    # contribution to y is the constant row Wo @ bv.
    corr = (Wo.astype(np.float64) @ bv.astype(np.float64)).astype(np.float32)
    y = np.empty((B, S, D), dtype=np.float32)
    for b in range(B):
        y[b] = (
            results[2 * b]["y"].astype(np.float32)
            + results[2 * b + 1]["y"].astype(np.float32)
            + corr
            + bo
        )
    return y


def _run(trace=False, **inputs):
    import time

    from concourse.bass_utils import run_bass_kernel_spmd

    nc = _build()
    args = {k: np.asarray(v) for k, v in inputs.items()}
    in_maps = _shard(**args)
    last_err = None
    for attempt in range(3):
        try:
            res = run_bass_kernel_spmd(
                nc, in_maps, core_ids=list(range(NCORES)), trace=trace
            )
            break
        except Exception as e:  # device occasionally wedges; retry recovers
            last_err = e
            time.sleep(10 * (attempt + 1))
    else:
        raise last_err
    y = _gather(res.results, args["Wv"], args["bv"], args["Wo"], args["bo"])
    return y, res


def kernel(**inputs):
    y, _ = _run(trace=False, **inputs)
    return y


# revision 26
# speedup vs baseline: 1.0275x; 1.0185x over previous
"""Multi-head attention (B=4, S=1024, D=1024, H=16) on 8 TRN2 NeuronCores.

Sharding: batch (4) x head-half (2) -> 8 cores, zero cross-core traffic.
Core c handles batch b = c // 2 and heads [hh*8, hh*8+8) where hh = c % 2.
Each core computes a partial output y_part[s, e] (its 512 channels fed
through its slice of Wo) in bf16; the host sums the two partials per batch
in fp32 and adds the bias terms.

v2 schedule (all bf16 matmuls, fp32 accumulation):
  - One strictly-ordered DMA chain on the sync queue: wq0,wk0, xq chunks,
    xk chunks, (xv,wv) chunks, remaining pair weights, wo.  Fine-grained
    per-chunk semaphores let projection matmuls start as chunks land.
  - Pair-0 Q/K projections are DMA-paced during the input load.
  - The 64 (pair, qn, kt) attention iterations run as a flat software
    pipeline: scores (row-tiled K=64 pair of matmuls) -> exp on ACT ->
    lag-2 AV accumulation, with a filler queue feeding the PE idle slots
    (V projection, next-pair Q/K projections, partial O-projection over
    pairs 0-2).  ACT does exp ONLY (normalize moved off it).
  - Normalize: 1/denom via DVE reciprocal_approx_fast on the psO ones-row,
    partition-broadcast + multiply on the Pool engine.
  - Tail: only the cc=3 O-projection matmuls + fused add with the
    cc0-2 partials, stored as bf16.
"""

import os
import sys

sys.path.insert(0, "/opt/trn_rl_repo")

import numpy as np
import ml_dtypes

BF16 = ml_dtypes.bfloat16

B, S, D = 4, 1024, 1024
HEADS = 16
DK = 64
P = 128
NCORES = 8
DCH = D // P       # 8 contraction chunks
PAIRS = 4          # head-pairs per core (8 heads / 2)
QN = 2             # q 512-chunks
KT = 8             # k tiles of 128
VW = 65            # V channels per head + ones column

_STATE = {}


def _build():
    """Build + compile the per-core Bass program (cached)."""
    if "nc" in _STATE:
        return _STATE["nc"]

    import concourse.bass as bass  # noqa: F401
    import concourse.mybir as mybir
    from concourse import bacc
    from concourse import tile

    f32 = mybir.dt.float32
    bf16 = mybir.dt.bfloat16
    AF = mybir.ActivationFunctionType
    ALU = mybir.AluOpType

    # Pin Exp to the one activation table containing it alongside Ln so the
    # table-load pass never alternates tables (each ACT_TABLE_LOAD ~1.3us).
    _orig_tables = bacc.get_activation_tables

    def _pinned_tables(arch):
        t = dict(_orig_tables(arch))
        target = "natural_log_exp_and_others"
        if target in t:
            for k in t:
                if k != target:
                    t[k] = t[k] - {AF.Exp, AF.Ln}
        return t

    bacc.get_activation_tables = _pinned_tables

    nc = bacc.Bacc("TRN2", target_bir_lowering=False, debug=False)

    xq_d = nc.dram_tensor("xq", [D, S], bf16, kind="ExternalInput")
    xk_d = nc.dram_tensor("xk", [D, S], bf16, kind="ExternalInput")
    xv_d = nc.dram_tensor("xv", [D, S], bf16, kind="ExternalInput")
    wq_d = nc.dram_tensor("wq", [PAIRS, D, P], bf16, kind="ExternalInput")
    wk_d = nc.dram_tensor("wk", [PAIRS, D, P], bf16, kind="ExternalInput")
    wv_d = nc.dram_tensor("wv", [D, 512], bf16, kind="ExternalInput")
    wo_d = nc.dram_tensor("wo", [512, D], bf16, kind="ExternalInput")
    bq_d = nc.dram_tensor("bq", [P, PAIRS], f32, kind="ExternalInput")
    bk_d = nc.dram_tensor("bk", [P, PAIRS], f32, kind="ExternalInput")
    mb_d = nc.dram_tensor("mb", [P, KT], f32, kind="ExternalInput")
    y_d = nc.dram_tensor("y", [S, D], bf16, kind="ExternalOutput")

    from contextlib import ExitStack

    with tile.TileContext(nc) as tc, ExitStack() as ctx:
        const = ctx.enter_context(tc.tile_pool(name="const", bufs=1))
        # Resident tensors
        wv_sb = const.tile([P, DCH, 512], bf16)
        xq_sb = const.tile([P, DCH, S], bf16)
        xk_sb = const.tile([P, DCH, S], bf16)
        xv_sb = const.tile([P, DCH, S], bf16)
        wo_sb = const.tile([P, PAIRS, D], bf16)
        v_sb = const.tile([P, KT, 8 * VW], bf16)
        cat_sb = const.tile([P, PAIRS, S], bf16)
        bq_sb = const.tile([P, PAIRS], f32)
        bk_sb = const.tile([P, PAIRS], f32)
        mb_sb = const.tile([P, KT], f32)

        # Pools
        wqp = ctx.enter_context(tc.tile_pool(name="wqp", bufs=3))
        wkp = ctx.enter_context(tc.tile_pool(name="wkp", bufs=3))
        qtp = ctx.enter_context(tc.tile_pool(name="qtp", bufs=2))
        ktp = ctx.enter_context(tc.tile_pool(name="ktp", bufs=2))
        epool = ctx.enter_context(tc.tile_pool(name="epool", bufs=14))
        spool = ctx.enter_context(tc.tile_pool(name="spool", bufs=3))
        rpool = ctx.enter_context(tc.tile_pool(name="rpool", bufs=4))
        r2pool = ctx.enter_context(tc.tile_pool(name="r2pool", bufs=3))
        ypool = ctx.enter_context(tc.tile_pool(name="ypool", bufs=3))
        psacc = ctx.enter_context(tc.tile_pool(name="psacc", bufs=2, space="PSUM"))
        pssp = ctx.enter_context(tc.tile_pool(name="pssp", bufs=2, space="PSUM"))
        psop = ctx.enter_context(tc.tile_pool(name="psop", bufs=2, space="PSUM"))

        # --- tiny loads + ones staging (off the main DMA chain) ---
        nc.scalar.dma_start(bq_sb[:], bq_d.ap())
        nc.scalar.dma_start(bk_sb[:], bk_d.ap())
        nc.scalar.dma_start(mb_sb[:], mb_d.ap())
        ones_f32 = const.tile([P, KT, 8], f32)
        nc.vector.memset(ones_f32[:], 1.0)
        ones_view = v_sb.rearrange("p t (h c) -> p t h c", c=VW)[:, :, :, 64:65]
        nc.vector.tensor_copy(ones_view, ones_f32[:].unsqueeze(3))

        # --- the ordered DMA chain (sync queue = strict transfer order) ---
        xq_r = xq_d.ap().rearrange("(d p) s -> d p s", p=P)
        xk_r = xk_d.ap().rearrange("(d p) s -> d p s", p=P)
        xv_r = xv_d.ap().rearrange("(d p) s -> d p s", p=P)
        wv_r = wv_d.ap().rearrange("(d p) m -> d p m", p=P)
        wq_r = wq_d.ap().rearrange("j (d p) m -> j p d m", p=P)
        wk_r = wk_d.ap().rearrange("j (d p) m -> j p d m", p=P)

        # Big-DMA chain on the gpsimd queue (idle until the first broadcast at
        # ~35us), in strict priority order.  Late pair weights + wo go on the
        # vector queue AFTER the pair-0 evicts so their transfers can't steal
        # HBM bandwidth from the critical xq/xk stream.
        wq_t = [None] * PAIRS
        wk_t = [None] * PAIRS
        wq_t[0] = wqp.tile([P, DCH, P], bf16, tag="wq", name="wq0")
        wk_t[0] = wkp.tile([P, DCH, P], bf16, tag="wk", name="wk0")
        nc.gpsimd.dma_start(wq_t[0][:], wq_r[0])
        nc.gpsimd.dma_start(wk_t[0][:], wk_r[0])
        for d in range(DCH):
            nc.gpsimd.dma_start(xq_sb[:, d], xq_r[d])
        for d in range(DCH):
            nc.gpsimd.dma_start(xk_sb[:, d], xk_r[d])
        for j in range(1, PAIRS):
            wq_t[j] = wqp.tile([P, DCH, P], bf16, tag="wq", name=f"wq{j}")
            wk_t[j] = wkp.tile([P, DCH, P], bf16, tag="wk", name=f"wk{j}")
        # wq1/wk1 right after xk so the pair-1 projection fillers can run
        # during pair 0; wq2/wq3/wo trail the xv stream (needed much later).
        nc.gpsimd.dma_start(wq_t[1][:], wq_r[1])
        nc.gpsimd.dma_start(wk_t[1][:], wk_r[1])
        for d in range(DCH):
            nc.gpsimd.dma_start(wv_sb[:, d], wv_r[d])
            nc.gpsimd.dma_start(xv_sb[:, d], xv_r[d])
        for j in range(2, PAIRS):
            nc.gpsimd.dma_start(wq_t[j][:], wq_r[j])
            nc.gpsimd.dma_start(wk_t[j][:], wk_r[j])
        nc.gpsimd.dma_start(wo_sb[:], wo_d.ap().rearrange("(c p) e -> p c e", p=P))

        qt_t = [None] * PAIRS
        kt_t = [None] * PAIRS

        def qk_group(proj, j, qn):
            """Generator: 8 DMA-paced projection matmuls + bias evict."""
            if proj == "q":
                if qt_t[j] is None:
                    qt_t[j] = qtp.tile([P, S], bf16, tag="qt", name=f"qt{j}")
                w, x, dst, b = wq_t[j], xq_sb, qt_t[j], bq_sb
            else:
                if kt_t[j] is None:
                    kt_t[j] = ktp.tile([P, S], bf16, tag="kt", name=f"kt{j}")
                w, x, dst, b = wk_t[j], xk_sb, kt_t[j], bk_sb
            ps = psacc.tile([P, 512], f32, tag="acc", name=f"ps{proj}{j}_{qn}")
            for d in range(DCH):
                nc.tensor.matmul(
                    ps[:],
                    w[:, d],
                    x[:, d, qn * 512 : (qn + 1) * 512],
                    start=(d == 0),
                    stop=(d == DCH - 1),
                )
                yield
            nc.vector.tensor_scalar_add(
                dst[:, qn * 512 : (qn + 1) * 512], ps[:], b[:, j : j + 1]
            )

        v_ready = [False] * KT

        def v_group(st0, nst):
            """Generator: V' projection for st0..st0+nst-1, d-interleaved."""
            ps = [
                psacc.tile([P, 512], f32, tag="acc", name=f"psv{st0 + i}")
                for i in range(nst)
            ]
            for d in range(DCH):
                for i in range(nst):
                    st = st0 + i
                    nc.tensor.matmul(
                        ps[i][:],
                        xv_sb[:, d, st * P : (st + 1) * P],
                        wv_sb[:, d],
                        start=(d == 0),
                        stop=(d == DCH - 1),
                    )
                    yield
            for i in range(nst):
                st = st0 + i
                vview = v_sb[:, st].rearrange("p (h c) -> p h c", c=VW)
                nc.vector.tensor_copy(
                    vview[:, :, 0:64], ps[i][:].rearrange("p (h c) -> p h c", c=64)
                )
                v_ready[st] = True

        # Pair-0 Q/K projections: DMA-paced, before the pipeline.
        for gen in (
            qk_group("q", 0, 0),
            qk_group("q", 0, 1),
            qk_group("k", 0, 0),
            qk_group("k", 0, 1),
        ):
            for _ in gen:
                pass



        # Filler queue for the attention pipeline: (min_slot, tag, generator).
        fillers = []
        fillers.append((2, "qk1", qk_group("q", 1, 0)))
        fillers.append((3, "qk1", qk_group("q", 1, 1)))
        fillers.append((2, "v", v_group(0, 2)))   # DMA-paced by xv arrival
        fillers.append((4, "v", v_group(2, 2)))
        fillers.append((6, "v", v_group(4, 2)))
        fillers.append((8, "v", v_group(6, 2)))
        fillers.append((8, "qk1", qk_group("k", 1, 0)))
        fillers.append((8, "qk1", qk_group("k", 1, 1)))
        for qn in range(QN):
            fillers.append((16 + 4 * qn, "qk2", qk_group("q", 2, qn)))
            fillers.append((20 + 4 * qn, "qk2", qk_group("k", 2, qn)))
        for qn in range(QN):
            fillers.append((32 + 4 * qn, "qk3", qk_group("q", 3, qn)))
            fillers.append((36 + 4 * qn, "qk3", qk_group("k", 3, qn)))
        y_r = y_d.ap().rearrange("(st p) e -> st p e", p=P)
        tail_n = [0]

        def o_final(st, en):
            """Full O-projection group (cc0-3) + copy evict + store."""
            i = tail_n[0]
            tail_n[0] += 1
            # During the pipeline (first half) stay off the scores pool —
            # stealing pssp there stalls the final pair's score tiles.
            if i < 8 or i % 4 < 2:
                ps = psacc.tile([P, 512], f32, tag="acc", name=f"psy2_{st}_{en}")[:]
            else:
                if i % 4 == 2:
                    o_final.pt = pssp.tile([P, 2, 512], f32, tag="s", name=f"psy2p_{st}_{en}")
                ps = o_final.pt[:, i % 2]
            for cc in range(PAIRS):
                nc.tensor.matmul(
                    ps,
                    cat_sb[:, cc, st * P : (st + 1) * P],
                    wo_sb[:, cc, en * 512 : (en + 1) * 512],
                    start=(cc == 0),
                    stop=(cc == PAIRS - 1),
                )
                yield
            y2 = ypool.tile([P, 512], bf16, tag="y", name=f"y{st}_{en}")
            if i < 8:
                nc.vector.tensor_copy(y2[:], ps)
            else:
                # post-exp tail: ACT is idle, evict there to unload DVE
                nc.scalar.activation(y2[:], ps, AF.Copy)
            nc.sync.dma_start(y_r[st][:, en * 512 : (en + 1) * 512], y2[:])

        # First half of the cc2/cc3 tail only needs cat3's qn0 range (s < 512),
        # whose normalize is emitted at slot 57 — run it as fillers.
        for st in range(4):
            for en in range(2):
                fillers.append((57, "t1", o_final(st, en)))
        total_filler_steps = 12 * 8 + 4 * 16 + 8 * 4  # MM emissions

        def pop_fillers(slot, budget):
            done = 0
            while done < budget and fillers:
                idx = next(
                    (i for i, (ms, _, _) in enumerate(fillers) if ms <= slot), None
                )
                if idx is None:
                    return done
                try:
                    next(fillers[idx][2])
                    done += 1
                except StopIteration:
                    fillers.pop(idx)
            return done

        def force_drain(tag):
            for entry in [f for f in fillers if f[1] == tag]:
                try:
                    while True:
                        next(entry[2])
                except StopIteration:
                    pass
                fillers.remove(entry)

        # --- the flat attention pipeline: 64 (j, qn, kt) iterations ---
        def emit_scores(j, qn, kt):
            pss = pssp.tile([P, 2, 512], f32, tag="s", name=f"pss{j}_{qn}_{kt}")
            for sub in range(2):
                lo, hi = sub * 64, (sub + 1) * 64
                nc.tensor.matmul(
                    pss[:, sub],
                    kt_t[j][lo:hi, kt * P : (kt + 1) * P],
                    qt_t[j][lo:hi, qn * 512 : (qn + 1) * 512],
                    start=True,
                    stop=True,
                )
            et = epool.tile([P, 2, 512], bf16, tag="e", name=f"e{j}_{qn}_{kt}")
            nc.scalar.activation(
                et[:], pss[:], AF.Exp, bias=mb_sb[:, kt : kt + 1], scale=1.0
            )
            return et

        pso_cur = {}

        def emit_av(j, qn, kt, et):
            for sub in range(2):
                h = j * 2 + sub
                if kt == 0:
                    pso_cur[sub] = psop.tile(
                        [VW, 512], f32, tag="o", name=f"pso{j}_{qn}_{sub}"
                    )
                nc.tensor.matmul(
                    pso_cur[sub][:],
                    v_sb[:, kt, h * VW : (h + 1) * VW],
                    et[:, sub],
                    start=(kt == 0),
                    stop=(kt == KT - 1),
                )

        def emit_normalize(j, qn):
            for sub in range(2):
                lo, hi = sub * 64, (sub + 1) * 64
                pso = pso_cur[sub]
                stg = spool.tile([64, 512], f32, tag="stg", name=f"stg{j}_{qn}_{sub}")
                nc.vector.tensor_copy(stg[:], pso[0:64, :])
                # 1/denom on DVE.  NOTE (HW-verified): reciprocal_approx_fast
                # silently corrupts unless its source sits at partition 0 in
                # SBUF, so the ones-row bounces through a partition-0 tile.
                # gpsimd runs ONLY partition_broadcast so its custom-op library
                # loads once (builtin ops there thrash LOAD_LIB, ~6.5us/swap).
                den = rpool.tile([1, 512], f32, tag="d", name=f"d{j}_{qn}_{sub}")
                nc.vector.tensor_copy(den[:], pso[64:65, :])
                rrow = rpool.tile([1, 512], f32, tag="r", name=f"r{j}_{qn}_{sub}")
                nc.vector.reciprocal_approx_fast(rrow[:], den[:])
                r2 = r2pool.tile([64, 512], f32, tag="r2", name=f"r2{j}_{qn}_{sub}")
                nc.gpsimd.partition_broadcast(r2[:], rrow[:])
                nc.vector.tensor_tensor(
                    cat_sb[lo:hi, j, qn * 512 : (qn + 1) * 512],
                    stg[0:64, :],
                    r2[:],
                    op=ALU.mult,
                )

        iters = [(j, qn, kt) for j in range(PAIRS) for qn in range(QN) for kt in range(KT)]
        pending = []  # (j, qn, kt, et) awaiting AV emission (lag-2)
        slot = 0
        remaining_steps = total_filler_steps

        def av_drain(n):
            # Pop pending AVs (in order) down to n, but never emit a pair-0 AV
            # before its v_sb k-tile write has been emitted (program-order RAW).
            while len(pending) > n:
                jj, qq, kk, ee = pending[0]
                if jj == 0 and not v_ready[kk]:
                    return
                pending.pop(0)
                emit_av(jj, qq, kk, ee)
                if kk == KT - 1:
                    emit_normalize(jj, qq)

        for j, qn, kt in iters:
            if j >= 1 and qn == 0 and kt == 0:
                force_drain(f"qk{j}")  # qt/kt writes must precede the reads
            et = emit_scores(j, qn, kt)
            pending.append((j, qn, kt, et))
            av_drain(2)
            budget = max(2, -(-remaining_steps // max(1, 64 - slot)))
            if len(pending) > 4:  # pair-0 backlog: push V emission along
                budget += len(pending) - 4
            remaining_steps -= pop_fillers(slot, budget)
            slot += 1
        av_drain(0)
        while fillers:
            if pop_fillers(10 ** 9, 1 << 30) == 0:
                break

        # --- tail: cc=3 O-projection for the remaining s-tiles ---
        for st in range(4, KT):
            for en in range(2):
                for _ in o_final(st, en):
                    pass

    nc.compile()
    _STATE["nc"] = nc
    return nc


def _shard(q, k, v, mask, Wq, bq, Wk, bk, Wv, bv, Wo, bo):
    """Build the 8 per-core input maps (host-side layout preparation)."""
    scale = 1.0 / np.sqrt(DK)
    in_maps = []
    for c in range(NCORES):
        b = c // 2
        hh = c % 2
        c0 = hh * 512
        wq_s = (Wq[c0 : c0 + 512, :] * scale).T  # [D, 512]
        wk_s = Wk[c0 : c0 + 512, :].T
        wv_s = Wv[c0 : c0 + 512, :].T
        wo_s = Wo[:, c0 : c0 + 512].T  # [512, D]
        mrow = mask[b, 0, 0, :]
        in_maps.append(
            {
                "xq": np.ascontiguousarray(q[b].T).astype(BF16),
                "xk": np.ascontiguousarray(k[b].T).astype(BF16),
                "xv": np.ascontiguousarray(v[b].T).astype(BF16),
                "wq": np.ascontiguousarray(
                    wq_s.reshape(D, PAIRS, P).transpose(1, 0, 2)
                ).astype(BF16),
                "wk": np.ascontiguousarray(
                    wk_s.reshape(D, PAIRS, P).transpose(1, 0, 2)
                ).astype(BF16),
                "wv": np.ascontiguousarray(wv_s).astype(BF16),
                "wo": np.ascontiguousarray(wo_s).astype(BF16),
                "bq": np.ascontiguousarray(
                    (bq[c0 : c0 + 512] * scale).reshape(PAIRS, P).T, dtype=np.float32
                ),
                "bk": np.ascontiguousarray(
                    bk[c0 : c0 + 512].reshape(PAIRS, P).T, dtype=np.float32
                ),
                "mb": np.ascontiguousarray(
                    np.where(mrow == 0, np.float32(-1e9), np.float32(0.0))
                    .astype(np.float32)
                    .reshape(KT, P)
                    .T
                ),
            }
        )
    return in_maps


def _gather(results, Wv, bv, Wo, bo):
    """Sum per-core partials into the full [B, S, D] output."""
    # Channel-bias correction folded out of the device kernel: the V bias
    # passes through softmax-weighted sums with total weight 1, so its
    # contribution to y is the constant row Wo @ bv.
    corr = (Wo.astype(np.float64) @ bv.astype(np.float64)).astype(np.float32)
    y = np.empty((B, S, D), dtype=np.float32)
    for b in range(B):
        y[b] = (
            results[2 * b]["y"].astype(np.float32)
            + results[2 * b + 1]["y"].astype(np.float32)
            + corr
            + bo
        )
    return y


def _run(trace=False, **inputs):
    import time

    from concourse.bass_utils import run_bass_kernel_spmd

    nc = _build()
    args = {k: np.asarray(v) for k, v in inputs.items()}
    in_maps = _shard(**args)
    last_err = None
    for attempt in range(3):
        try:
            res = run_bass_kernel_spmd(
                nc, in_maps, core_ids=list(range(NCORES)), trace=trace
            )
            break
        except Exception as e:  # device occasionally wedges; retry recovers
            last_err = e
            time.sleep(10 * (attempt + 1))
    else:
        raise last_err
    y = _gather(res.results, args["Wv"], args["bv"], args["Wo"], args["bo"])
    return y, res


def kernel(**inputs):
    y, _ = _run(trace=False, **inputs)
    return y


# revision 27
# speedup vs baseline: 1.0500x; 1.0219x over previous
"""Multi-head attention (B=4, S=1024, D=1024, H=16) on 8 TRN2 NeuronCores.

Sharding: batch (4) x head-half (2) -> 8 cores, zero cross-core traffic.
Core c handles batch b = c // 2 and heads [hh*8, hh*8+8) where hh = c % 2.
Each core computes a partial output y_part[s, e] (its 512 channels fed
through its slice of Wo) in bf16; the host sums the two partials per batch
in fp32 and adds the bias terms.

v2 schedule (all bf16 matmuls, fp32 accumulation):
  - One strictly-ordered DMA chain on the sync queue: wq0,wk0, xq chunks,
    xk chunks, (xv,wv) chunks, remaining pair weights, wo.  Fine-grained
    per-chunk semaphores let projection matmuls start as chunks land.
  - Pair-0 Q/K projections are DMA-paced during the input load.
  - The 64 (pair, qn, kt) attention iterations run as a flat software
    pipeline: scores (row-tiled K=64 pair of matmuls) -> exp on ACT ->
    lag-2 AV accumulation, with a filler queue feeding the PE idle slots
    (V projection, next-pair Q/K projections, partial O-projection over
    pairs 0-2).  ACT does exp ONLY (normalize moved off it).
  - Normalize: 1/denom via DVE reciprocal_approx_fast on the psO ones-row,
    partition-broadcast + multiply on the Pool engine.
  - Tail: only the cc=3 O-projection matmuls + fused add with the
    cc0-2 partials, stored as bf16.
"""

import os
import sys

sys.path.insert(0, "/opt/trn_rl_repo")

import numpy as np
import ml_dtypes

BF16 = ml_dtypes.bfloat16

B, S, D = 4, 1024, 1024
HEADS = 16
DK = 64
P = 128
NCORES = 8
DCH = D // P       # 8 contraction chunks
PAIRS = 4          # head-pairs per core (8 heads / 2)
QN = 2             # q 512-chunks
KT = 8             # k tiles of 128
VW = 65            # V channels per head + ones column

_STATE = {}


def _build():
    """Build + compile the per-core Bass program (cached)."""
    if "nc" in _STATE:
        return _STATE["nc"]

    import concourse.bass as bass  # noqa: F401
    import concourse.mybir as mybir
    from concourse import bacc
    from concourse import tile

    f32 = mybir.dt.float32
    bf16 = mybir.dt.bfloat16
    AF = mybir.ActivationFunctionType
    ALU = mybir.AluOpType

    # Pin Exp to the one activation table containing it alongside Ln so the
    # table-load pass never alternates tables (each ACT_TABLE_LOAD ~1.3us).
    _orig_tables = bacc.get_activation_tables

    def _pinned_tables(arch):
        t = dict(_orig_tables(arch))
        target = "natural_log_exp_and_others"
        if target in t:
            for k in t:
                if k != target:
                    t[k] = t[k] - {AF.Exp, AF.Ln}
        return t

    bacc.get_activation_tables = _pinned_tables

    nc = bacc.Bacc("TRN2", target_bir_lowering=False, debug=False)

    xq_d = nc.dram_tensor("xq", [D, S], bf16, kind="ExternalInput")
    xk_d = nc.dram_tensor("xk", [D, S], bf16, kind="ExternalInput")
    xv_d = nc.dram_tensor("xv", [D, S], bf16, kind="ExternalInput")
    wq_d = nc.dram_tensor("wq", [PAIRS, D, P], bf16, kind="ExternalInput")
    wk_d = nc.dram_tensor("wk", [PAIRS, D, P], bf16, kind="ExternalInput")
    wv_d = nc.dram_tensor("wv", [D, 512], bf16, kind="ExternalInput")
    wo_d = nc.dram_tensor("wo", [512, D], bf16, kind="ExternalInput")
    bq_d = nc.dram_tensor("bq", [P, PAIRS], f32, kind="ExternalInput")
    bk_d = nc.dram_tensor("bk", [P, PAIRS], f32, kind="ExternalInput")
    mb_d = nc.dram_tensor("mb", [P, KT], f32, kind="ExternalInput")
    y_d = nc.dram_tensor("y", [S, D], bf16, kind="ExternalOutput")

    from contextlib import ExitStack

    with tile.TileContext(nc) as tc, ExitStack() as ctx:
        const = ctx.enter_context(tc.tile_pool(name="const", bufs=1))
        # Resident tensors
        wv_sb = const.tile([P, DCH, 512], bf16)
        xq_sb = const.tile([P, DCH, S], bf16)
        xk_sb = const.tile([P, DCH, S], bf16)
        xv_sb = const.tile([P, DCH, S], bf16)
        wo_sb = const.tile([P, PAIRS, D], bf16)
        v_sb = const.tile([P, KT, 8 * VW], bf16)
        cat_sb = const.tile([P, PAIRS, S], bf16)
        bq_sb = const.tile([P, PAIRS], f32)
        bk_sb = const.tile([P, PAIRS], f32)
        mb_sb = const.tile([P, KT], f32)

        # Pools
        wqp = ctx.enter_context(tc.tile_pool(name="wqp", bufs=3))
        wkp = ctx.enter_context(tc.tile_pool(name="wkp", bufs=3))
        qtp = ctx.enter_context(tc.tile_pool(name="qtp", bufs=2))
        ktp = ctx.enter_context(tc.tile_pool(name="ktp", bufs=2))
        epool = ctx.enter_context(tc.tile_pool(name="epool", bufs=14))
        spool = ctx.enter_context(tc.tile_pool(name="spool", bufs=3))
        rpool = ctx.enter_context(tc.tile_pool(name="rpool", bufs=4))
        r2pool = ctx.enter_context(tc.tile_pool(name="r2pool", bufs=3))
        ypool = ctx.enter_context(tc.tile_pool(name="ypool", bufs=3))
        psacc = ctx.enter_context(tc.tile_pool(name="psacc", bufs=2, space="PSUM"))
        pssp = ctx.enter_context(tc.tile_pool(name="pssp", bufs=2, space="PSUM"))
        psop = ctx.enter_context(tc.tile_pool(name="psop", bufs=2, space="PSUM"))

        # --- tiny loads + ones staging (off the main DMA chain) ---
        nc.scalar.dma_start(bq_sb[:], bq_d.ap())
        nc.scalar.dma_start(bk_sb[:], bk_d.ap())
        nc.scalar.dma_start(mb_sb[:], mb_d.ap())
        ones_f32 = const.tile([P, KT, 8], f32)
        nc.vector.memset(ones_f32[:], 1.0)
        ones_view = v_sb.rearrange("p t (h c) -> p t h c", c=VW)[:, :, :, 64:65]
        nc.vector.tensor_copy(ones_view, ones_f32[:].unsqueeze(3))

        # --- the ordered DMA chain (sync queue = strict transfer order) ---
        xq_r = xq_d.ap().rearrange("(d p) s -> d p s", p=P)
        xk_r = xk_d.ap().rearrange("(d p) s -> d p s", p=P)
        xv_r = xv_d.ap().rearrange("(d p) s -> d p s", p=P)
        wv_r = wv_d.ap().rearrange("(d p) m -> d p m", p=P)
        wq_r = wq_d.ap().rearrange("j (d p) m -> j p d m", p=P)
        wk_r = wk_d.ap().rearrange("j (d p) m -> j p d m", p=P)

        # Big-DMA chain on the gpsimd queue (idle until the first broadcast at
        # ~35us), in strict priority order.  Late pair weights + wo go on the
        # vector queue AFTER the pair-0 evicts so their transfers can't steal
        # HBM bandwidth from the critical xq/xk stream.
        wq_t = [None] * PAIRS
        wk_t = [None] * PAIRS
        wq_t[0] = wqp.tile([P, DCH, P], bf16, tag="wq", name="wq0")
        wk_t[0] = wkp.tile([P, DCH, P], bf16, tag="wk", name="wk0")
        nc.gpsimd.dma_start(wq_t[0][:], wq_r[0])
        nc.gpsimd.dma_start(wk_t[0][:], wk_r[0])
        for d in range(DCH):
            nc.gpsimd.dma_start(xq_sb[:, d], xq_r[d])
        for d in range(DCH):
            nc.gpsimd.dma_start(xk_sb[:, d], xk_r[d])
        for j in range(1, PAIRS):
            wq_t[j] = wqp.tile([P, DCH, P], bf16, tag="wq", name=f"wq{j}")
            wk_t[j] = wkp.tile([P, DCH, P], bf16, tag="wk", name=f"wk{j}")
        # wq1/wk1 right after xk so the pair-1 projection fillers can run
        # during pair 0; wq2/wq3/wo trail the xv stream (needed much later).
        nc.gpsimd.dma_start(wq_t[1][:], wq_r[1])
        nc.gpsimd.dma_start(wk_t[1][:], wk_r[1])
        for d in range(DCH):
            nc.gpsimd.dma_start(wv_sb[:, d], wv_r[d])
            nc.gpsimd.dma_start(xv_sb[:, d], xv_r[d])
        for j in range(2, PAIRS):
            nc.gpsimd.dma_start(wq_t[j][:], wq_r[j])
            nc.gpsimd.dma_start(wk_t[j][:], wk_r[j])
        nc.gpsimd.dma_start(wo_sb[:], wo_d.ap().rearrange("(c p) e -> p c e", p=P))

        qt_t = [None] * PAIRS
        kt_t = [None] * PAIRS

        def qk_group(proj, j, qn):
            """Generator: 8 DMA-paced projection matmuls + bias evict."""
            if proj == "q":
                if qt_t[j] is None:
                    qt_t[j] = qtp.tile([P, S], bf16, tag="qt", name=f"qt{j}")
                w, x, dst, b = wq_t[j], xq_sb, qt_t[j], bq_sb
            else:
                if kt_t[j] is None:
                    kt_t[j] = ktp.tile([P, S], bf16, tag="kt", name=f"kt{j}")
                w, x, dst, b = wk_t[j], xk_sb, kt_t[j], bk_sb
            ps = psacc.tile([P, 512], f32, tag="acc", name=f"ps{proj}{j}_{qn}")
            for d in range(DCH):
                nc.tensor.matmul(
                    ps[:],
                    w[:, d],
                    x[:, d, qn * 512 : (qn + 1) * 512],
                    start=(d == 0),
                    stop=(d == DCH - 1),
                )
                yield
            nc.vector.tensor_scalar_add(
                dst[:, qn * 512 : (qn + 1) * 512], ps[:], b[:, j : j + 1]
            )

        v_ready = [False] * KT

        def v_group(st0, nst):
            """Generator: V' projection for st0..st0+nst-1, d-interleaved."""
            ps = [
                psacc.tile([P, 512], f32, tag="acc", name=f"psv{st0 + i}")
                for i in range(nst)
            ]
            for d in range(DCH):
                for i in range(nst):
                    st = st0 + i
                    nc.tensor.matmul(
                        ps[i][:],
                        xv_sb[:, d, st * P : (st + 1) * P],
                        wv_sb[:, d],
                        start=(d == 0),
                        stop=(d == DCH - 1),
                    )
                    yield
            for i in range(nst):
                st = st0 + i
                vview = v_sb[:, st].rearrange("p (h c) -> p h c", c=VW)
                nc.vector.tensor_copy(
                    vview[:, :, 0:64], ps[i][:].rearrange("p (h c) -> p h c", c=64)
                )
                v_ready[st] = True

        # Pair-0 Q/K projections: DMA-paced, before the pipeline.
        for gen in (
            qk_group("q", 0, 0),
            qk_group("q", 0, 1),
            qk_group("k", 0, 0),
            qk_group("k", 0, 1),
        ):
            for _ in gen:
                pass



        # Filler queue for the attention pipeline: (min_slot, tag, generator).
        fillers = []
        fillers.append((2, "qk1", qk_group("q", 1, 0)))
        fillers.append((3, "qk1", qk_group("q", 1, 1)))
        fillers.append((2, "v", v_group(0, 2)))   # DMA-paced by xv arrival
        fillers.append((4, "v", v_group(2, 2)))
        fillers.append((6, "v", v_group(4, 2)))
        fillers.append((8, "v", v_group(6, 2)))
        fillers.append((8, "qk1", qk_group("k", 1, 0)))
        fillers.append((8, "qk1", qk_group("k", 1, 1)))
        for qn in range(QN):
            fillers.append((16 + 4 * qn, "qk2", qk_group("q", 2, qn)))
            fillers.append((20 + 4 * qn, "qk2", qk_group("k", 2, qn)))
        for qn in range(QN):
            fillers.append((32 + 4 * qn, "qk3", qk_group("q", 3, qn)))
            fillers.append((36 + 4 * qn, "qk3", qk_group("k", 3, qn)))
        y_r = y_d.ap().rearrange("(st p) e -> st p e", p=P)
        tail_n = [0]

        def o_final(st, en):
            """Full O-projection group (cc0-3) + copy evict + store."""
            i = tail_n[0]
            tail_n[0] += 1
            # During the pipeline (first half) stay off the scores pool —
            # stealing pssp there stalls the final pair's score tiles.
            if i < 8 or i % 4 < 2:
                ps = psacc.tile([P, 512], f32, tag="acc", name=f"psy2_{st}_{en}")[:]
            else:
                if i % 4 == 2:
                    o_final.pt = pssp.tile([P, 2, 512], f32, tag="s", name=f"psy2p_{st}_{en}")
                ps = o_final.pt[:, i % 2]
            for cc in range(PAIRS):
                nc.tensor.matmul(
                    ps,
                    cat_sb[:, cc, st * P : (st + 1) * P],
                    wo_sb[:, cc, en * 512 : (en + 1) * 512],
                    start=(cc == 0),
                    stop=(cc == PAIRS - 1),
                )
                yield
            y2 = ypool.tile([P, 512], bf16, tag="y", name=f"y{st}_{en}")
            # Evict on ACT: keeps DVE free for the final pair's normalize
            # chain (stg/recip/mult), which gates the whole tail.
            nc.scalar.activation(y2[:], ps, AF.Copy)
            nc.sync.dma_start(y_r[st][:, en * 512 : (en + 1) * 512], y2[:])

        # First half of the cc2/cc3 tail only needs cat3's qn0 range (s < 512),
        # whose normalize is emitted at slot 57 — run it as fillers.
        for st in range(4):
            for en in range(2):
                fillers.append((57, "t1", o_final(st, en)))
        total_filler_steps = 12 * 8 + 4 * 16 + 8 * 4  # MM emissions

        def pop_fillers(slot, budget):
            done = 0
            while done < budget and fillers:
                idx = next(
                    (i for i, (ms, _, _) in enumerate(fillers) if ms <= slot), None
                )
                if idx is None:
                    return done
                try:
                    next(fillers[idx][2])
                    done += 1
                except StopIteration:
                    fillers.pop(idx)
            return done

        def force_drain(tag):
            for entry in [f for f in fillers if f[1] == tag]:
                try:
                    while True:
                        next(entry[2])
                except StopIteration:
                    pass
                fillers.remove(entry)

        # --- the flat attention pipeline: 64 (j, qn, kt) iterations ---
        def emit_scores(j, qn, kt):
            pss = pssp.tile([P, 2, 512], f32, tag="s", name=f"pss{j}_{qn}_{kt}")
            for sub in range(2):
                lo, hi = sub * 64, (sub + 1) * 64
                nc.tensor.matmul(
                    pss[:, sub],
                    kt_t[j][lo:hi, kt * P : (kt + 1) * P],
                    qt_t[j][lo:hi, qn * 512 : (qn + 1) * 512],
                    start=True,
                    stop=True,
                )
            et = epool.tile([P, 2, 512], bf16, tag="e", name=f"e{j}_{qn}_{kt}")
            nc.scalar.activation(
                et[:], pss[:], AF.Exp, bias=mb_sb[:, kt : kt + 1], scale=1.0
            )
            return et

        pso_cur = {}

        def emit_av(j, qn, kt, et):
            for sub in range(2):
                h = j * 2 + sub
                if kt == 0:
                    pso_cur[sub] = psop.tile(
                        [VW, 512], f32, tag="o", name=f"pso{j}_{qn}_{sub}"
                    )
                nc.tensor.matmul(
                    pso_cur[sub][:],
                    v_sb[:, kt, h * VW : (h + 1) * VW],
                    et[:, sub],
                    start=(kt == 0),
                    stop=(kt == KT - 1),
                )

        def emit_normalize(j, qn):
            for sub in range(2):
                lo, hi = sub * 64, (sub + 1) * 64
                pso = pso_cur[sub]
                stg = spool.tile([64, 512], f32, tag="stg", name=f"stg{j}_{qn}_{sub}")
                nc.vector.tensor_copy(stg[:], pso[0:64, :])
                # 1/denom on DVE.  NOTE (HW-verified): reciprocal_approx_fast
                # silently corrupts unless its source sits at partition 0 in
                # SBUF, so the ones-row bounces through a partition-0 tile.
                # gpsimd runs ONLY partition_broadcast so its custom-op library
                # loads once (builtin ops there thrash LOAD_LIB, ~6.5us/swap).
                den = rpool.tile([1, 512], f32, tag="d", name=f"d{j}_{qn}_{sub}")
                nc.vector.tensor_copy(den[:], pso[64:65, :])
                rrow = rpool.tile([1, 512], f32, tag="r", name=f"r{j}_{qn}_{sub}")
                nc.vector.reciprocal_approx_fast(rrow[:], den[:])
                r2 = r2pool.tile([64, 512], f32, tag="r2", name=f"r2{j}_{qn}_{sub}")
                nc.gpsimd.partition_broadcast(r2[:], rrow[:])
                nc.vector.tensor_tensor(
                    cat_sb[lo:hi, j, qn * 512 : (qn + 1) * 512],
                    stg[0:64, :],
                    r2[:],
                    op=ALU.mult,
                )

        iters = [(j, qn, kt) for j in range(PAIRS) for qn in range(QN) for kt in range(KT)]
        pending = []  # (j, qn, kt, et) awaiting AV emission (lag-2)
        slot = 0
        remaining_steps = total_filler_steps

        def av_drain(n):
            # Pop pending AVs (in order) down to n, but never emit a pair-0 AV
            # before its v_sb k-tile write has been emitted (program-order RAW).
            while len(pending) > n:
                jj, qq, kk, ee = pending[0]
                if jj == 0 and not v_ready[kk]:
                    return
                pending.pop(0)
                emit_av(jj, qq, kk, ee)
                if kk == KT - 1:
                    emit_normalize(jj, qq)

        for j, qn, kt in iters:
            if j >= 1 and qn == 0 and kt == 0:
                force_drain(f"qk{j}")  # qt/kt writes must precede the reads
            et = emit_scores(j, qn, kt)
            pending.append((j, qn, kt, et))
            av_drain(2)
            budget = max(2, -(-remaining_steps // max(1, 64 - slot)))
            if len(pending) > 4:  # pair-0 backlog: push V emission along
                budget += len(pending) - 4
            remaining_steps -= pop_fillers(slot, budget)
            slot += 1
        av_drain(0)
        while fillers:
            if pop_fillers(10 ** 9, 1 << 30) == 0:
                break

        # --- tail: cc=3 O-projection for the remaining s-tiles ---
        for st in range(4, KT):
            for en in range(2):
                for _ in o_final(st, en):
                    pass

    nc.compile()
    _STATE["nc"] = nc
    return nc


def _shard(q, k, v, mask, Wq, bq, Wk, bk, Wv, bv, Wo, bo):
    """Build the 8 per-core input maps (host-side layout preparation)."""
    scale = 1.0 / np.sqrt(DK)
    in_maps = []
    for c in range(NCORES):
        b = c // 2
        hh = c % 2
        c0 = hh * 512
        wq_s = (Wq[c0 : c0 + 512, :] * scale).T  # [D, 512]
        wk_s = Wk[c0 : c0 + 512, :].T
        wv_s = Wv[c0 : c0 + 512, :].T
        wo_s = Wo[:, c0 : c0 + 512].T  # [512, D]
        mrow = mask[b, 0, 0, :]
        in_maps.append(
            {
                "xq": np.ascontiguousarray(q[b].T).astype(BF16),
                "xk": np.ascontiguousarray(k[b].T).astype(BF16),
                "xv": np.ascontiguousarray(v[b].T).astype(BF16),
                "wq": np.ascontiguousarray(
                    wq_s.reshape(D, PAIRS, P).transpose(1, 0, 2)
                ).astype(BF16),
                "wk": np.ascontiguousarray(
                    wk_s.reshape(D, PAIRS, P).transpose(1, 0, 2)
                ).astype(BF16),
                "wv": np.ascontiguousarray(wv_s).astype(BF16),
                "wo": np.ascontiguousarray(wo_s).astype(BF16),
                "bq": np.ascontiguousarray(
                    (bq[c0 : c0 + 512] * scale).reshape(PAIRS, P).T, dtype=np.float32
                ),
                "bk": np.ascontiguousarray(
                    bk[c0 : c0 + 512].reshape(PAIRS, P).T, dtype=np.float32
                ),
                "mb": np.ascontiguousarray(
                    np.where(mrow == 0, np.float32(-1e9), np.float32(0.0))
                    .astype(np.float32)
                    .reshape(KT, P)
                    .T
                ),
            }
        )
    return in_maps


def _gather(results, Wv, bv, Wo, bo):
    """Sum per-core partials into the full [B, S, D] output."""
    # Channel-bias correction folded out of the device kernel: the V bias
    # passes through softmax-weighted sums with total weight 1, so its
    # contribution to y is the constant row Wo @ bv.
    corr = (Wo.astype(np.float64) @ bv.astype(np.float64)).astype(np.float32)
    y = np.empty((B, S, D), dtype=np.float32)
    for b in range(B):
        y[b] = (
            results[2 * b]["y"].astype(np.float32)
            + results[2 * b + 1]["y"].astype(np.float32)
            + corr
            + bo
        )
    return y


def _run(trace=False, **inputs):
    import time

    from concourse.bass_utils import run_bass_kernel_spmd

    nc = _build()
    args = {k: np.asarray(v) for k, v in inputs.items()}
    in_maps = _shard(**args)
    last_err = None
    for attempt in range(3):
        try:
            res = run_bass_kernel_spmd(
                nc, in_maps, core_ids=list(range(NCORES)), trace=trace
            )
            break
        except Exception as e:  # device occasionally wedges; retry recovers
            last_err = e
            time.sleep(10 * (attempt + 1))
    else:
        raise last_err
    y = _gather(res.results, args["Wv"], args["bv"], args["Wo"], args["bo"])
    return y, res


def kernel(**inputs):
    y, _ = _run(trace=False, **inputs)
    return y


# revision 28
# speedup vs baseline: 1.0811x; 1.0295x over previous
"""Multi-head attention (B=4, S=1024, D=1024, H=16) on 8 TRN2 NeuronCores.

Sharding: batch (4) x head-half (2) -> 8 cores, zero cross-core traffic.
Core c handles batch b = c // 2 and heads [hh*8, hh*8+8) where hh = c % 2.
Each core computes a partial output y_part[s, e] (its 512 channels fed
through its slice of Wo) in bf16; the host sums the two partials per batch
in fp32 and adds the bias terms.

v2 schedule (all bf16 matmuls, fp32 accumulation):
  - One strictly-ordered DMA chain on the sync queue: wq0,wk0, xq chunks,
    xk chunks, (xv,wv) chunks, remaining pair weights, wo.  Fine-grained
    per-chunk semaphores let projection matmuls start as chunks land.
  - Pair-0 Q/K projections are DMA-paced during the input load.
  - The 64 (pair, qn, kt) attention iterations run as a flat software
    pipeline: scores (row-tiled K=64 pair of matmuls) -> exp on ACT ->
    lag-2 AV accumulation, with a filler queue feeding the PE idle slots
    (V projection, next-pair Q/K projections, partial O-projection over
    pairs 0-2).  ACT does exp ONLY (normalize moved off it).
  - Normalize: 1/denom via DVE reciprocal_approx_fast on the psO ones-row,
    partition-broadcast + multiply on the Pool engine.
  - Tail: only the cc=3 O-projection matmuls + fused add with the
    cc0-2 partials, stored as bf16.
"""

import os
import sys

sys.path.insert(0, "/opt/trn_rl_repo")

import numpy as np
import ml_dtypes

BF16 = ml_dtypes.bfloat16

B, S, D = 4, 1024, 1024
HEADS = 16
DK = 64
P = 128
NCORES = 8
DCH = D // P       # 8 contraction chunks
PAIRS = 4          # head-pairs per core (8 heads / 2)
QN = 2             # q 512-chunks
KT = 8             # k tiles of 128
VW = 65            # V channels per head + ones column

_STATE = {}


def _build():
    """Build + compile the per-core Bass program (cached)."""
    if "nc" in _STATE:
        return _STATE["nc"]

    import concourse.bass as bass  # noqa: F401
    import concourse.mybir as mybir
    from concourse import bacc
    from concourse import tile

    f32 = mybir.dt.float32
    bf16 = mybir.dt.bfloat16
    AF = mybir.ActivationFunctionType
    ALU = mybir.AluOpType

    # Pin Exp to the one activation table containing it alongside Ln so the
    # table-load pass never alternates tables (each ACT_TABLE_LOAD ~1.3us).
    _orig_tables = bacc.get_activation_tables

    def _pinned_tables(arch):
        t = dict(_orig_tables(arch))
        target = "natural_log_exp_and_others"
        if target in t:
            for k in t:
                if k != target:
                    t[k] = t[k] - {AF.Exp, AF.Ln}
        return t

    bacc.get_activation_tables = _pinned_tables

    nc = bacc.Bacc("TRN2", target_bir_lowering=False, debug=False)

    xq_d = nc.dram_tensor("xq", [D, S], bf16, kind="ExternalInput")
    xk_d = nc.dram_tensor("xk", [D, S], bf16, kind="ExternalInput")
    xv_d = nc.dram_tensor("xv", [D, S], bf16, kind="ExternalInput")
    wq_d = nc.dram_tensor("wq", [PAIRS, D, P], bf16, kind="ExternalInput")
    wk_d = nc.dram_tensor("wk", [PAIRS, D, P], bf16, kind="ExternalInput")
    wv_d = nc.dram_tensor("wv", [D, 512], bf16, kind="ExternalInput")
    wo_d = nc.dram_tensor("wo", [512, D], bf16, kind="ExternalInput")
    bq_d = nc.dram_tensor("bq", [P, PAIRS], f32, kind="ExternalInput")
    bk_d = nc.dram_tensor("bk", [P, PAIRS], f32, kind="ExternalInput")
    mb_d = nc.dram_tensor("mb", [P, KT], f32, kind="ExternalInput")
    y_d = nc.dram_tensor("y", [S, D], bf16, kind="ExternalOutput")

    from contextlib import ExitStack

    with tile.TileContext(nc) as tc, ExitStack() as ctx:
        const = ctx.enter_context(tc.tile_pool(name="const", bufs=1))
        # Resident tensors
        wv_sb = const.tile([P, DCH, 512], bf16)
        xq_sb = const.tile([P, DCH, S], bf16)
        xk_sb = const.tile([P, DCH, S], bf16)
        xv_sb = const.tile([P, DCH, S], bf16)
        wo_sb = const.tile([P, PAIRS, D], bf16)
        v_sb = const.tile([P, KT, 8 * VW], bf16)
        cat_sb = const.tile([P, PAIRS, S], bf16)
        bq_sb = const.tile([P, PAIRS], f32)
        bk_sb = const.tile([P, PAIRS], f32)
        mb_sb = const.tile([P, KT], f32)

        # Pools
        wqp = ctx.enter_context(tc.tile_pool(name="wqp", bufs=3))
        wkp = ctx.enter_context(tc.tile_pool(name="wkp", bufs=3))
        qtp = ctx.enter_context(tc.tile_pool(name="qtp", bufs=2))
        ktp = ctx.enter_context(tc.tile_pool(name="ktp", bufs=2))
        epool = ctx.enter_context(tc.tile_pool(name="epool", bufs=14))
        spool = ctx.enter_context(tc.tile_pool(name="spool", bufs=3))
        rpool = ctx.enter_context(tc.tile_pool(name="rpool", bufs=4))
        r2pool = ctx.enter_context(tc.tile_pool(name="r2pool", bufs=3))
        ypool = ctx.enter_context(tc.tile_pool(name="ypool", bufs=4))
        psacc = ctx.enter_context(tc.tile_pool(name="psacc", bufs=2, space="PSUM"))
        pssp = ctx.enter_context(tc.tile_pool(name="pssp", bufs=2, space="PSUM"))
        psop = ctx.enter_context(tc.tile_pool(name="psop", bufs=2, space="PSUM"))

        # --- tiny loads + ones staging (off the main DMA chain) ---
        nc.scalar.dma_start(bq_sb[:], bq_d.ap())
        nc.scalar.dma_start(bk_sb[:], bk_d.ap())
        nc.scalar.dma_start(mb_sb[:], mb_d.ap())
        ones_f32 = const.tile([P, KT, 8], f32)
        nc.vector.memset(ones_f32[:], 1.0)
        ones_view = v_sb.rearrange("p t (h c) -> p t h c", c=VW)[:, :, :, 64:65]
        nc.vector.tensor_copy(ones_view, ones_f32[:].unsqueeze(3))

        # --- the ordered DMA chain (sync queue = strict transfer order) ---
        xq_r = xq_d.ap().rearrange("(d p) s -> d p s", p=P)
        xk_r = xk_d.ap().rearrange("(d p) s -> d p s", p=P)
        xv_r = xv_d.ap().rearrange("(d p) s -> d p s", p=P)
        wv_r = wv_d.ap().rearrange("(d p) m -> d p m", p=P)
        wq_r = wq_d.ap().rearrange("j (d p) m -> j p d m", p=P)
        wk_r = wk_d.ap().rearrange("j (d p) m -> j p d m", p=P)

        # Big-DMA chain on the gpsimd queue (idle until the first broadcast at
        # ~35us), in strict priority order.  Late pair weights + wo go on the
        # vector queue AFTER the pair-0 evicts so their transfers can't steal
        # HBM bandwidth from the critical xq/xk stream.
        wq_t = [None] * PAIRS
        wk_t = [None] * PAIRS
        wq_t[0] = wqp.tile([P, DCH, P], bf16, tag="wq", name="wq0")
        wk_t[0] = wkp.tile([P, DCH, P], bf16, tag="wk", name="wk0")
        # First weights on the scalar queue: its preamble drains first, so
        # these transfers start ~2us before the gpsimd chain's first chunk.
        nc.scalar.dma_start(wq_t[0][:], wq_r[0])
        nc.scalar.dma_start(wk_t[0][:], wk_r[0])
        for d in range(DCH):
            nc.gpsimd.dma_start(xq_sb[:, d], xq_r[d])
        for d in range(DCH):
            nc.gpsimd.dma_start(xk_sb[:, d], xk_r[d])
        for j in range(1, PAIRS):
            wq_t[j] = wqp.tile([P, DCH, P], bf16, tag="wq", name=f"wq{j}")
            wk_t[j] = wkp.tile([P, DCH, P], bf16, tag="wk", name=f"wk{j}")
        # wq1/wk1 right after xk so the pair-1 projection fillers can run
        # during pair 0; wq2/wq3/wo trail the xv stream (needed much later).
        nc.gpsimd.dma_start(wq_t[1][:], wq_r[1])
        nc.gpsimd.dma_start(wk_t[1][:], wk_r[1])
        for d in range(DCH):
            nc.gpsimd.dma_start(wv_sb[:, d], wv_r[d])
            nc.gpsimd.dma_start(xv_sb[:, d], xv_r[d])
        for j in range(2, PAIRS):
            nc.gpsimd.dma_start(wq_t[j][:], wq_r[j])
            nc.gpsimd.dma_start(wk_t[j][:], wk_r[j])
        nc.gpsimd.dma_start(wo_sb[:], wo_d.ap().rearrange("(c p) e -> p c e", p=P))

        qt_t = [None] * PAIRS
        kt_t = [None] * PAIRS

        def qk_group(proj, j, qn):
            """Generator: 8 DMA-paced projection matmuls + bias evict."""
            if proj == "q":
                if qt_t[j] is None:
                    qt_t[j] = qtp.tile([P, S], bf16, tag="qt", name=f"qt{j}")
                w, x, dst, b = wq_t[j], xq_sb, qt_t[j], bq_sb
            else:
                if kt_t[j] is None:
                    kt_t[j] = ktp.tile([P, S], bf16, tag="kt", name=f"kt{j}")
                w, x, dst, b = wk_t[j], xk_sb, kt_t[j], bk_sb
            ps = psacc.tile([P, 512], f32, tag="acc", name=f"ps{proj}{j}_{qn}")
            for d in range(DCH):
                nc.tensor.matmul(
                    ps[:],
                    w[:, d],
                    x[:, d, qn * 512 : (qn + 1) * 512],
                    start=(d == 0),
                    stop=(d == DCH - 1),
                )
                yield
            nc.vector.tensor_scalar_add(
                dst[:, qn * 512 : (qn + 1) * 512], ps[:], b[:, j : j + 1]
            )

        v_ready = [False] * KT

        def v_group(st0, nst):
            """Generator: V' projection for st0..st0+nst-1, d-interleaved."""
            ps = [
                psacc.tile([P, 512], f32, tag="acc", name=f"psv{st0 + i}")
                for i in range(nst)
            ]
            for d in range(DCH):
                for i in range(nst):
                    st = st0 + i
                    nc.tensor.matmul(
                        ps[i][:],
                        xv_sb[:, d, st * P : (st + 1) * P],
                        wv_sb[:, d],
                        start=(d == 0),
                        stop=(d == DCH - 1),
                    )
                    yield
            for i in range(nst):
                st = st0 + i
                vview = v_sb[:, st].rearrange("p (h c) -> p h c", c=VW)
                nc.vector.tensor_copy(
                    vview[:, :, 0:64], ps[i][:].rearrange("p (h c) -> p h c", c=64)
                )
                v_ready[st] = True

        # Pair-0 Q/K projections: DMA-paced, before the pipeline.
        for gen in (
            qk_group("q", 0, 0),
            qk_group("q", 0, 1),
            qk_group("k", 0, 0),
            qk_group("k", 0, 1),
        ):
            for _ in gen:
                pass



        # Filler queue for the attention pipeline: (min_slot, tag, generator).
        fillers = []
        fillers.append((2, "qk1", qk_group("q", 1, 0)))
        fillers.append((3, "qk1", qk_group("q", 1, 1)))
        fillers.append((2, "v", v_group(0, 2)))   # DMA-paced by xv arrival
        fillers.append((4, "v", v_group(2, 2)))
        fillers.append((6, "v", v_group(4, 2)))
        fillers.append((8, "v", v_group(6, 2)))
        fillers.append((8, "qk1", qk_group("k", 1, 0)))
        fillers.append((8, "qk1", qk_group("k", 1, 1)))
        for qn in range(QN):
            fillers.append((16 + 4 * qn, "qk2", qk_group("q", 2, qn)))
            fillers.append((20 + 4 * qn, "qk2", qk_group("k", 2, qn)))
        for qn in range(QN):
            fillers.append((32 + 4 * qn, "qk3", qk_group("q", 3, qn)))
            fillers.append((36 + 4 * qn, "qk3", qk_group("k", 3, qn)))
        y_r = y_d.ap().rearrange("(st p) e -> st p e", p=P)
        tail_n = [0]

        def o_final(st, en):
            """Full O-projection group (cc0-3) + copy evict + store."""
            i = tail_n[0]
            tail_n[0] += 1
            # During the pipeline (first half) stay off the scores pool —
            # stealing pssp there stalls the final pair's score tiles.
            if i < 8 or i % 4 < 2:
                ps = psacc.tile([P, 512], f32, tag="acc", name=f"psy2_{st}_{en}")[:]
            else:
                if i % 4 == 2:
                    o_final.pt = pssp.tile([P, 2, 512], f32, tag="s", name=f"psy2p_{st}_{en}")
                ps = o_final.pt[:, i % 2]
            for cc in range(PAIRS):
                nc.tensor.matmul(
                    ps,
                    cat_sb[:, cc, st * P : (st + 1) * P],
                    wo_sb[:, cc, en * 512 : (en + 1) * 512],
                    start=(cc == 0),
                    stop=(cc == PAIRS - 1),
                )
                yield
            y2 = ypool.tile([P, 512], bf16, tag="y", name=f"y{st}_{en}")
            # Evict on ACT: keeps DVE free for the final pair's normalize
            # chain (stg/recip/mult), which gates the whole tail.
            nc.scalar.activation(y2[:], ps, AF.Copy)
            nc.sync.dma_start(y_r[st][:, en * 512 : (en + 1) * 512], y2[:])

        # First half of the cc2/cc3 tail only needs cat3's qn0 range (s < 512),
        # whose normalize is emitted at slot 57 — run it as fillers.
        for st in range(4):
            for en in range(2):
                fillers.append((57, "t1", o_final(st, en)))
        total_filler_steps = 12 * 8 + 4 * 16 + 8 * 4  # MM emissions

        def pop_fillers(slot, budget):
            done = 0
            while done < budget and fillers:
                idx = next(
                    (i for i, (ms, _, _) in enumerate(fillers) if ms <= slot), None
                )
                if idx is None:
                    return done
                try:
                    next(fillers[idx][2])
                    done += 1
                except StopIteration:
                    fillers.pop(idx)
            return done

        def force_drain(tag):
            for entry in [f for f in fillers if f[1] == tag]:
                try:
                    while True:
                        next(entry[2])
                except StopIteration:
                    pass
                fillers.remove(entry)

        # --- the flat attention pipeline: 64 (j, qn, kt) iterations ---
        def emit_scores(j, qn, kt):
            pss = pssp.tile([P, 2, 512], f32, tag="s", name=f"pss{j}_{qn}_{kt}")
            for sub in range(2):
                lo, hi = sub * 64, (sub + 1) * 64
                nc.tensor.matmul(
                    pss[:, sub],
                    kt_t[j][lo:hi, kt * P : (kt + 1) * P],
                    qt_t[j][lo:hi, qn * 512 : (qn + 1) * 512],
                    start=True,
                    stop=True,
                )
            et = epool.tile([P, 2, 512], bf16, tag="e", name=f"e{j}_{qn}_{kt}")
            nc.scalar.activation(
                et[:], pss[:], AF.Exp, bias=mb_sb[:, kt : kt + 1], scale=1.0
            )
            return et

        pso_cur = {}

        def emit_av(j, qn, kt, et):
            for sub in range(2):
                h = j * 2 + sub
                if kt == 0:
                    pso_cur[sub] = psop.tile(
                        [VW, 512], f32, tag="o", name=f"pso{j}_{qn}_{sub}"
                    )
                nc.tensor.matmul(
                    pso_cur[sub][:],
                    v_sb[:, kt, h * VW : (h + 1) * VW],
                    et[:, sub],
                    start=(kt == 0),
                    stop=(kt == KT - 1),
                )

        def emit_normalize(j, qn):
            for sub in range(2):
                lo, hi = sub * 64, (sub + 1) * 64
                pso = pso_cur[sub]
                stg = spool.tile([64, 512], f32, tag="stg", name=f"stg{j}_{qn}_{sub}")
                nc.vector.tensor_copy(stg[:], pso[0:64, :])
                # 1/denom on DVE.  NOTE (HW-verified): reciprocal_approx_fast
                # silently corrupts unless its source sits at partition 0 in
                # SBUF, so the ones-row bounces through a partition-0 tile.
                # gpsimd runs ONLY partition_broadcast so its custom-op library
                # loads once (builtin ops there thrash LOAD_LIB, ~6.5us/swap).
                den = rpool.tile([1, 512], f32, tag="d", name=f"d{j}_{qn}_{sub}")
                nc.vector.tensor_copy(den[:], pso[64:65, :])
                rrow = rpool.tile([1, 512], f32, tag="r", name=f"r{j}_{qn}_{sub}")
                nc.vector.reciprocal_approx_fast(rrow[:], den[:])
                r2 = r2pool.tile([64, 512], f32, tag="r2", name=f"r2{j}_{qn}_{sub}")
                nc.gpsimd.partition_broadcast(r2[:], rrow[:])
                nc.vector.tensor_tensor(
                    cat_sb[lo:hi, j, qn * 512 : (qn + 1) * 512],
                    stg[0:64, :],
                    r2[:],
                    op=ALU.mult,
                )

        iters = [(j, qn, kt) for j in range(PAIRS) for qn in range(QN) for kt in range(KT)]
        pending = []  # (j, qn, kt, et) awaiting AV emission (lag-2)
        slot = 0
        remaining_steps = total_filler_steps

        def av_drain(n):
            # Pop pending AVs (in order) down to n, but never emit a pair-0 AV
            # before its v_sb k-tile write has been emitted (program-order RAW).
            while len(pending) > n:
                jj, qq, kk, ee = pending[0]
                if jj == 0 and not v_ready[kk]:
                    return
                pending.pop(0)
                emit_av(jj, qq, kk, ee)
                if kk == KT - 1:
                    emit_normalize(jj, qq)

        for j, qn, kt in iters:
            if j >= 1 and qn == 0 and kt == 0:
                force_drain(f"qk{j}")  # qt/kt writes must precede the reads
            et = emit_scores(j, qn, kt)
            pending.append((j, qn, kt, et))
            av_drain(2)
            budget = max(2, -(-remaining_steps // max(1, 64 - slot)))
            if len(pending) > 4:  # pair-0 backlog: push V emission along
                budget += len(pending) - 4
            remaining_steps -= pop_fillers(slot, budget)
            slot += 1
        av_drain(0)
        while fillers:
            if pop_fillers(10 ** 9, 1 << 30) == 0:
                break

        # --- tail: cc=3 O-projection for the remaining s-tiles ---
        for st in range(4, KT):
            for en in range(2):
                for _ in o_final(st, en):
                    pass

    nc.compile()
    _STATE["nc"] = nc
    return nc


def _shard(q, k, v, mask, Wq, bq, Wk, bk, Wv, bv, Wo, bo):
    """Build the 8 per-core input maps (host-side layout preparation)."""
    scale = 1.0 / np.sqrt(DK)
    in_maps = []
    for c in range(NCORES):
        b = c // 2
        hh = c % 2
        c0 = hh * 512
        wq_s = (Wq[c0 : c0 + 512, :] * scale).T  # [D, 512]
        wk_s = Wk[c0 : c0 + 512, :].T
        wv_s = Wv[c0 : c0 + 512, :].T
        wo_s = Wo[:, c0 : c0 + 512].T  # [512, D]
        mrow = mask[b, 0, 0, :]
        in_maps.append(
            {
                "xq": np.ascontiguousarray(q[b].T).astype(BF16),
                "xk": np.ascontiguousarray(k[b].T).astype(BF16),
                "xv": np.ascontiguousarray(v[b].T).astype(BF16),
                "wq": np.ascontiguousarray(
                    wq_s.reshape(D, PAIRS, P).transpose(1, 0, 2)
                ).astype(BF16),
                "wk": np.ascontiguousarray(
                    wk_s.reshape(D, PAIRS, P).transpose(1, 0, 2)
                ).astype(BF16),
                "wv": np.ascontiguousarray(wv_s).astype(BF16),
                "wo": np.ascontiguousarray(wo_s).astype(BF16),
                "bq": np.ascontiguousarray(
                    (bq[c0 : c0 + 512] * scale).reshape(PAIRS, P).T, dtype=np.float32
                ),
                "bk": np.ascontiguousarray(
                    bk[c0 : c0 + 512].reshape(PAIRS, P).T, dtype=np.float32
                ),
                "mb": np.ascontiguousarray(
                    np.where(mrow == 0, np.float32(-1e9), np.float32(0.0))
                    .astype(np.float32)
                    .reshape(KT, P)
                    .T
                ),
            }
        )
    return in_maps


def _gather(results, Wv, bv, Wo, bo):
    """Sum per-core partials into the full [B, S, D] output."""
    # Channel-bias correction folded out of the device kernel: the V bias
    # passes through softmax-weighted sums with total weight 1, so its
    # contribution to y is the constant row Wo @ bv.
    corr = (Wo.astype(np.float64) @ bv.astype(np.float64)).astype(np.float32)
    y = np.empty((B, S, D), dtype=np.float32)
    for b in range(B):
        y[b] = (
            results[2 * b]["y"].astype(np.float32)
            + results[2 * b + 1]["y"].astype(np.float32)
            + corr
            + bo
        )
    return y


def _run(trace=False, **inputs):
    import time

    from concourse.bass_utils import run_bass_kernel_spmd

    nc = _build()
    args = {k: np.asarray(v) for k, v in inputs.items()}
    in_maps = _shard(**args)
    last_err = None
    for attempt in range(3):
        try:
            res = run_bass_kernel_spmd(
                nc, in_maps, core_ids=list(range(NCORES)), trace=trace
            )
            break
        except Exception as e:  # device occasionally wedges; retry recovers
            last_err = e
            time.sleep(10 * (attempt + 1))
    else:
        raise last_err
    y = _gather(res.results, args["Wv"], args["bv"], args["Wo"], args["bo"])
    return y, res


def kernel(**inputs):
    y, _ = _run(trace=False, **inputs)
    return y
